# revision 1
# baseline (speedup 1.0000x reference)
"""ConvSA kernel for Trainium2 (8 NeuronCores, data-parallel over batch).

Computes, per batch element b (one per core):
    q/k/v = conv3x3(feat, W{q,k,v}) + b{q,k,v}        # 256 -> 512 ch, SAME pad
    att   = softmax_j(q^T k);  out = v @ att^T + v    # N = 48*48 = 2304

Strategy: all matmuls in float32r (full-rate fp32 storage, ~13-bit
mantissa inputs). Convs as 18 accumulated matmuls (2 c-chunks x 9 taps)
over a zero-padded [128, 2, 50, 50] SBUF image. Attention computed in the
s^T[j, i] orientation (both QK operands in natural conv-output layout),
with a single global shift constant C (column max of the first 128 i's)
instead of per-row max -- mathematically identical softmax, safe in fp32.
p = exp(s - C) stays unnormalized; rowsums via ones-vector matmul;
normalization folded into the output epilogue.
"""
import numpy as np
from contextlib import ExitStack

import concourse.bass as bass
import concourse.tile as tile
from concourse import bacc, bass_utils, mybir
from concourse.masks import make_identity

F32 = mybir.dt.float32
F32R = mybir.dt.float32r

B, C, H, W = 8, 256, 48, 48
E = 512
N = H * W            # 2304
CC = C // 128        # 2 c-chunks
OC = E // 128        # 4 o-chunks / e-chunks
JC = N // 128        # 18 j-chunks
NT = [(0, 10), (10, 10), (20, 10), (30, 10), (40, 8)]     # conv row tiles
IT = [(0, 512), (512, 512), (1024, 512), (1536, 512), (2048, 256)]  # i tiles

_CACHE = {}


def _build():
    nc = bacc.Bacc("TRN2", target_bir_lowering=False, debug=False, num_devices=B)

    xp_ap = nc.dram_tensor("xpad", [128, CC, 2500], F32R, kind="ExternalInput").ap()
    w_aps = {
        cn: nc.dram_tensor(f"w{cn}", [OC, 128, CC, 9, 128], F32R, kind="ExternalInput").ap()
        for cn in "qkv"
    }
    b_aps = {
        cn: nc.dram_tensor(f"b{cn}", [128, OC], F32, kind="ExternalInput").ap()
        for cn in "qkv"
    }
    out_ap = nc.dram_tensor("out", [OC, 128, N], F32, kind="ExternalOutput").ap()

    with tile.TileContext(nc) as tc, ExitStack() as ctx:
        res = ctx.enter_context(tc.tile_pool(name="res", bufs=1))
        k_res = res.tile([128, OC, N], F32R, tag="k")
        q_res = res.tile([128, OC, N], F32R, tag="q")
        vT = res.tile([128, JC, E], F32R, tag="vT")
        bias_t = {cn: res.tile([128, OC], F32, tag=f"b{cn}", name=f"bias_{cn}")
                  for cn in "qkv"}
        ones_col = res.tile([128, 1], F32R, tag="oc")
        ones_row = res.tile([1, 128], F32R, tag="or")
        negC = res.tile([128, 1], F32, tag="negc")
        ident = res.tile([128, 128], F32R, tag="id")

        dram = ctx.enter_context(tc.tile_pool(name="dram", bufs=1, space="DRAM"))
        v_scr = dram.tile([OC, 128, N], F32R)

        for cn in "qkv":
            nc.sync.dma_start(out=bias_t[cn], in_=b_aps[cn])

        # ---------------- conv phase ----------------
        with tc.tile_pool(name="xw", bufs=1) as xwp, \
             tc.tile_pool(name="w", bufs=3) as wp, \
             tc.tile_pool(name="vst", bufs=2) as vstp, \
             tc.tile_pool(name="cps", bufs=2, space="PSUM") as cps:
            ident_raw = xwp.tile([128, 128], F32, tag="idr")
            make_identity(nc, ident_raw)
            nc.vector.tensor_copy(out=ident, in_=ident_raw)
            ones_raw = xwp.tile([128, 1], F32, tag="onr")
            nc.vector.memset(ones_raw, 1.0)
            nc.vector.tensor_copy(out=ones_col, in_=ones_raw)
            ones_raw2 = xwp.tile([1, 128], F32, tag="onr2")
            nc.vector.memset(ones_raw2, 1.0)
            nc.vector.tensor_copy(out=ones_row, in_=ones_raw2)
            xpad_t = xwp.tile([128, CC, 50, 50], F32R, tag="x")
            nc.sync.dma_start(
                out=xpad_t.rearrange("p c h w -> p c (h w)"), in_=xp_ap
            )

            def conv(cn, sink):
                for oc in range(OC):
                    w_t = wp.tile([128, CC, 9, 128], F32R, tag="w")
                    nc.sync.dma_start(out=w_t, in_=w_aps[cn][oc])
                    for (y0, rr) in NT:
                        ps = cps.tile([128, rr * 48], F32, tag="conv")
                        first = True
                        for cc in range(CC):
                            for ky in range(3):
                                for kx in range(3):
                                    rhs = xpad_t[:, cc, y0 + ky:y0 + ky + rr, kx:kx + 48]
                                    nc.tensor.matmul(
                                        ps, w_t[:, cc, ky * 3 + kx, :], rhs,
                                        start=first, stop=(cc == CC - 1 and ky == 2 and kx == 2),
                                    )
                                    first = False
                        sink(cn, oc, y0, rr, ps)

            def to_res(dst):
                def sink(cn, oc, y0, rr, ps):
                    nc.scalar.activation(
                        out=dst[:, oc, y0 * 48:(y0 + rr) * 48], in_=ps,
                        func=mybir.ActivationFunctionType.Identity,
                        bias=bias_t[cn][:, oc:oc + 1], scale=1.0,
                    )
                return sink

            conv("k", to_res(k_res))
            conv("q", to_res(q_res))

            # ---- global shift constant C (hidden under V conv) ----
            # C = max over i in [0,256) x j in [0,1024) of s -- any constant
            # with  rowmax-80 <= C <= globalmax+88  keeps exp() in fp32 range,
            # and softmax is shift-invariant so the result is exact.
            with tc.tile_pool(name="mps", bufs=1, space="PSUM") as mps, \
                 tc.tile_pool(name="msb", bufs=1) as msb, \
                 tc.tile_pool(name="nps", bufs=1, space="PSUM") as nps:
                mini = mps.tile([128, 8, 256], F32)
                for jc in range(8):
                    for ec in range(OC):
                        nc.tensor.matmul(
                            mini[:, jc, :], k_res[:, ec, jc * 128:(jc + 1) * 128],
                            q_res[:, ec, 0:256], start=(ec == 0), stop=(ec == OC - 1),
                        )
                m1 = msb.tile([128, 1], F32R, tag="m1")
                nc.vector.reduce_max(out=m1, in_=mini, axis=mybir.AxisListType.XY)
                tpm = nps.tile([1, 128], F32R, tag="tp")
                nc.tensor.transpose(tpm, m1, ident)
                cneg = msb.tile([1, 2], F32R, tag="cn")
                nc.vector.reduce_max(out=cneg[:, 0:1], in_=tpm,
                                     axis=mybir.AxisListType.X, negate=True)
                nc.vector.tensor_copy(out=cneg[:, 1:2], in_=cneg[:, 0:1])
                ncps = nps.tile([128, 2], F32, tag="ncps")
                nc.tensor.matmul(ncps, ones_row, cneg, start=True, stop=True)
                nc.vector.tensor_copy(out=negC, in_=ncps[:, 0:1])

            # v conv: stage per o-chunk, DMA to scratch + transpose into vT
            with tc.tile_pool(name="tps", bufs=2, space="PSUM") as tps:
                for oc in range(OC):
                    w_t = wp.tile([128, CC, 9, 128], F32R, tag="w")
                    nc.sync.dma_start(out=w_t, in_=w_aps["v"][oc])
                    vs = vstp.tile([128, N], F32R, tag="vs")
                    for (y0, rr) in NT:
                        ps = cps.tile([128, rr * 48], F32, tag="conv")
                        first = True
                        for cc in range(CC):
                            for ky in range(3):
                                for kx in range(3):
                                    rhs = xpad_t[:, cc, y0 + ky:y0 + ky + rr, kx:kx + 48]
                                    nc.tensor.matmul(
                                        ps, w_t[:, cc, ky * 3 + kx, :], rhs,
                                        start=first, stop=(cc == CC - 1 and ky == 2 and kx == 2),
                                    )
                                    first = False
                        nc.scalar.activation(
                            out=vs[:, y0 * 48:(y0 + rr) * 48], in_=ps,
                            func=mybir.ActivationFunctionType.Identity,
                            bias=bias_t["v"][:, oc:oc + 1], scale=1.0,
                        )
                    nc.sync.dma_start(out=v_scr[oc], in_=vs)
                    for jc in range(JC):
                        tp = tps.tile([128, 128], F32R, tag="t")
                        nc.tensor.transpose(tp, vs[:, jc * 128:(jc + 1) * 128], ident)
                        nc.vector.tensor_copy(out=vT[:, jc, oc * 128:(oc + 1) * 128], in_=tp)

        # ---------------- attention ----------------
        with tc.tile_pool(name="pp", bufs=2) as pp, \
             tc.tile_pool(name="esb", bufs=2) as esb, \
             tc.tile_pool(name="sps", bufs=3, space="PSUM") as sps, \
             tc.tile_pool(name="aps", bufs=2, space="PSUM") as aps, \
             tc.tile_pool(name="rps", bufs=1, space="PSUM") as rps, \
             tc.tile_pool(name="bps", bufs=1, space="PSUM") as bps:
            p_tiles = {}

            def emit_qk(t):
                i0, iw = IT[t]
                p_t = pp.tile([128, JC, iw], F32R, tag="p")
                p_tiles[t] = p_t
                for jc in range(JC):
                    ps = sps.tile([128, iw], F32, tag="s")
                    for ec in range(OC):
                        nc.tensor.matmul(
                            ps, k_res[:, ec, jc * 128:(jc + 1) * 128],
                            q_res[:, ec, i0:i0 + iw],
                            start=(ec == 0), stop=(ec == OC - 1),
                        )
                    nc.scalar.activation(
                        out=p_t[:, jc, :], in_=ps,
                        func=mybir.ActivationFunctionType.Exp,
                        bias=negC[:, 0:1], scale=1.0,
                    )

            def emit_post(t):
                i0, iw = IT[t]
                p_t = p_tiles.pop(t)
                rs = rps.tile([1, iw], F32, tag="rs")
                for jc in range(JC):
                    nc.tensor.matmul(rs, ones_col, p_t[:, jc, :],
                                     start=(jc == 0), stop=(jc == JC - 1))
                r_sb = esb.tile([1, iw], F32R, tag="r")
                with nc.allow_low_precision(reason="f32r recip feeds f32r matmul"):
                    nc.vector.reciprocal(out=r_sb, in_=rs)
                avs = []
                for ec in range(OC):
                    av = aps.tile([128, iw], F32, tag="av", name=f"av_{t}_{ec}")
                    for jc in range(JC):
                        nc.tensor.matmul(
                            av, vT[:, jc, ec * 128:(ec + 1) * 128], p_t[:, jc, :],
                            start=(jc == 0), stop=(jc == JC - 1),
                        )
                    avs.append(av)
                rbc = bps.tile([128, iw], F32, tag="rbc")
                nc.tensor.matmul(rbc, ones_row, r_sb, start=True, stop=True)
                rbc_sb = esb.tile([128, iw], F32, tag="rbcs")
                nc.vector.tensor_copy(out=rbc_sb, in_=rbc)
                for ec in range(OC):
                    vs_t = esb.tile([128, iw], F32R, tag="vstream", name=f"vst_{t}_{ec}")
                    nc.sync.dma_start(out=vs_t, in_=v_scr[ec, :, i0:i0 + iw])
                    o_t = esb.tile([128, iw], F32, tag="o", name=f"o_{t}_{ec}")
                    nc.vector.tensor_tensor(o_t, avs[ec], rbc_sb, mybir.AluOpType.mult)
                    nc.vector.tensor_tensor(o_t, o_t, vs_t, mybir.AluOpType.add)
                    nc.sync.dma_start(out=out_ap[ec, :, i0:i0 + iw], in_=o_t)

            emit_qk(0)
            for t in range(1, len(IT)):
                emit_qk(t)
                emit_post(t - 1)
            emit_post(len(IT) - 1)

    nc.compile()
    return nc


def _prep_shared(Wq, bq, Wk, bk, Wv, bv):
    def wprep(Wm):
        A = Wm.reshape(OC, 128, CC, 128, 3, 3)
        Bm = A.transpose(0, 3, 2, 4, 5, 1)      # [oc, c, cc, ky, kx, o]
        return np.ascontiguousarray(Bm.reshape(OC, 128, CC, 9, 128), dtype=np.float32)

    def bprep(bm):
        return np.ascontiguousarray(bm.reshape(OC, 128).T, dtype=np.float32)

    return {
        "wq": wprep(Wq), "wk": wprep(Wk), "wv": wprep(Wv),
        "bq": bprep(bq), "bk": bprep(bk), "bv": bprep(bv),
    }


def kernel(feat, Wq, bq, Wk, bk, Wv, bv):
    feat = np.asarray(feat, dtype=np.float32)
    if "nc" not in _CACHE:
        _CACHE["nc"] = _build()
    nc = _CACHE["nc"]

    shared = _prep_shared(np.asarray(Wq, np.float32), np.asarray(bq, np.float32),
                          np.asarray(Wk, np.float32), np.asarray(bk, np.float32),
                          np.asarray(Wv, np.float32), np.asarray(bv, np.float32))

    in_maps = []
    for b in range(B):
        xpad = np.zeros((C, 50, 50), np.float32)
        xpad[:, 1:49, 1:49] = feat[b]
        xpad = np.ascontiguousarray(
            xpad.reshape(CC, 128, 2500).transpose(1, 0, 2)
        )
        in_maps.append({"xpad": xpad, **shared})

    r = bass_utils.run_bass_kernel_spmd(nc, in_maps, list(range(B)))
    out = np.stack(
        [r.results[b]["out"].reshape(E, H, W) for b in range(B)], axis=0
    )
    return out



# revision 5
# speedup vs baseline: 1.1402x; 1.1402x over previous
"""ConvSA kernel for Trainium2 (8 NeuronCores, data-parallel over batch).

Computes, per batch element b (one per core):
    q/k/v = conv3x3(feat, W{q,k,v}) + b{q,k,v}        # 256 -> 512 ch, SAME pad
    att   = softmax_j(q^T k);  out = v @ att^T + v    # N = 48*48 = 2304

v2 strategy vs v1 (460us): fp16 operands for convs + QK (LDWEIGHTS at
1 cy/row makes matmuls stream-bound instead of weight-load-bound), bf16
for p=exp(s-C) and vT (bf16 keeps fp32 exponent range; fp16 would
underflow rows whose max logit is far below the global shift C).
Attention-value matmul is flipped to produce out^T tiles [i_part, e]:
p slices are the stationary operand, vT streams; the epilogue then
normalizes with a per-partition scalar (activation scale), adds the
residual v^T directly from vT (v_scr DRAM roundtrip eliminated), and
row-sums come from a DVE pairwise tree + one ones-matmul instead of 18
PE matmuls per tile. Output is written transposed [N, E]; the host
untransposes.
"""
import numpy as np
from contextlib import ExitStack

import concourse.bass as bass
import concourse.tile as tile
from concourse import bacc, bass_utils, mybir
from concourse.masks import make_identity

F32 = mybir.dt.float32
F32R = mybir.dt.float32r
F16 = mybir.dt.float16
BF16 = mybir.dt.bfloat16

B, C, H, W = 8, 256, 48, 48
E = 512
N = H * W            # 2304
CC = C // 128        # 2 c-chunks
OC = E // 128        # 4 o-chunks / e-chunks
JC = N // 128        # 18 j-chunks
NT = [(0, 10), (10, 10), (20, 10), (30, 10), (40, 8)]     # conv row tiles
IT = [(0, 512), (512, 512), (1024, 512), (1536, 512), (2048, 256)]  # i tiles

_CACHE = {}


def _build():
    nc = bacc.Bacc("TRN2", target_bir_lowering=False, debug=False, num_devices=B)

    xp_ap = nc.dram_tensor("xpad", [128, CC, 2500], F16, kind="ExternalInput").ap()
    w_aps = {
        cn: nc.dram_tensor(f"w{cn}", [OC, 128, CC, 9, 128], F16, kind="ExternalInput").ap()
        for cn in "qkv"
    }
    b_aps = {
        cn: nc.dram_tensor(f"b{cn}", [128, OC], F32, kind="ExternalInput").ap()
        for cn in "qkv"
    }
    # transposed output: [i-chunk, 128 i, E]
    out_ap = nc.dram_tensor("out", [JC, 128, E], F32, kind="ExternalOutput").ap()

    with tile.TileContext(nc) as tc, ExitStack() as ctx:
        res = ctx.enter_context(tc.tile_pool(name="res", bufs=1))
        k_res = res.tile([128, OC, N], F16, tag="k")
        q_res = res.tile([128, OC, N], F16, tag="q")
        vT = res.tile([128, JC, E], BF16, tag="vT")
        bias_t = {cn: res.tile([128, OC], F32, tag=f"b{cn}", name=f"bias_{cn}")
                  for cn in "qkv"}
        ones_col = res.tile([128, 1], F32R, tag="oc")
        ones_row = res.tile([1, 128], F32R, tag="or")
        ones_one = res.tile([1, 2], F32R, tag="o1")
        negC = res.tile([128, 1], F32, tag="negc")
        ident = res.tile([128, 128], F32R, tag="id")

        # ---------------- conv phase ----------------
        with tc.tile_pool(name="xw", bufs=1) as xwp, \
             tc.tile_pool(name="w", bufs=3) as wp, \
             tc.tile_pool(name="vst", bufs=2) as vstp, \
             tc.tile_pool(name="cps", bufs=2, space="PSUM") as cps:
            xpad_t = xwp.tile([128, CC, 50, 50], F16, tag="x")
            # split DMA per c-chunk: first conv matmuls need only cc0
            for cc in range(CC):
                nc.sync.dma_start(
                    out=xpad_t[:, cc].rearrange("p h w -> p (h w)"),
                    in_=xp_ap[:, cc],
                )
            ident_raw = xwp.tile([128, 128], F32, tag="idr")
            make_identity(nc, ident_raw)
            nc.vector.tensor_copy(out=ident, in_=ident_raw)
            ones_raw = xwp.tile([128, 1], F32, tag="onr")
            nc.vector.memset(ones_raw, 1.0)
            nc.vector.tensor_copy(out=ones_col, in_=ones_raw)
            ones_raw2 = xwp.tile([1, 128], F32, tag="onr2")
            nc.vector.memset(ones_raw2, 1.0)
            nc.vector.tensor_copy(out=ones_row, in_=ones_raw2)
            nc.vector.tensor_copy(out=ones_one, in_=ones_raw2[:, 0:2])
            for cn in "qkv":
                nc.sync.dma_start(out=bias_t[cn], in_=b_aps[cn])

            def conv(cn, sink, out_dt):
                for oc in range(OC):
                    w_t = wp.tile([128, CC, 9, 128], F16, tag="w")
                    nc.sync.dma_start(out=w_t, in_=w_aps[cn][oc])
                    for (y0, rr) in NT:
                        ps = cps.tile([128, rr * 48], F32, tag="conv")
                        first = True
                        for cc in range(CC):
                            for ky in range(3):
                                for kx in range(3):
                                    rhs = xpad_t[:, cc, y0 + ky:y0 + ky + rr, kx:kx + 48]
                                    nc.tensor.matmul(
                                        ps, w_t[:, cc, ky * 3 + kx, :], rhs,
                                        start=first, stop=(cc == CC - 1 and ky == 2 and kx == 2),
                                    )
                                    first = False
                        sink(cn, oc, y0, rr, ps)

            def to_res(dst):
                def sink(cn, oc, y0, rr, ps):
                    nc.scalar.activation(
                        out=dst[:, oc, y0 * 48:(y0 + rr) * 48], in_=ps,
                        func=mybir.ActivationFunctionType.Identity,
                        bias=bias_t[cn][:, oc:oc + 1], scale=1.0,
                    )
                return sink

            conv("k", to_res(k_res), F16)
            conv("q", to_res(q_res), F16)

            # ---- global shift constant C (hidden under V conv) ----
            # C = max over i in [0,256) x j in [0,1024) of s -- any constant
            # with  rowmax-80 <= C <= globalmax+88  keeps exp() in fp32/bf16
            # range, and softmax is shift-invariant so the result is exact.
            with tc.tile_pool(name="mps", bufs=1, space="PSUM") as mps, \
                 tc.tile_pool(name="msb", bufs=1) as msb, \
                 tc.tile_pool(name="nps", bufs=1, space="PSUM") as nps:
                mini = mps.tile([128, 8, 256], F32)
                for jc in range(8):
                    for ec in range(OC):
                        nc.tensor.matmul(
                            mini[:, jc, :], k_res[:, ec, jc * 128:(jc + 1) * 128],
                            q_res[:, ec, 0:256], start=(ec == 0), stop=(ec == OC - 1),
                        )
                m1 = msb.tile([128, 1], F32R, tag="m1")
                nc.vector.reduce_max(out=m1, in_=mini, axis=mybir.AxisListType.XY)
                tpm = nps.tile([1, 128], F32R, tag="tp")
                nc.tensor.transpose(tpm, m1, ident)
                cneg = msb.tile([1, 2], F32R, tag="cn")
                nc.vector.reduce_max(out=cneg[:, 0:1], in_=tpm,
                                     axis=mybir.AxisListType.X, negate=True)
                nc.vector.tensor_copy(out=cneg[:, 1:2], in_=cneg[:, 0:1])
                ncps = nps.tile([128, 2], F32, tag="ncps")
                nc.tensor.matmul(ncps, ones_row, cneg, start=True, stop=True)
                nc.vector.tensor_copy(out=negC, in_=ncps[:, 0:1])

            # v conv: per o-chunk, transpose into vT (bf16); residual v^T
            # is read straight out of vT later, no DRAM scratch needed.
            with tc.tile_pool(name="tps", bufs=2, space="PSUM") as tps:
                for oc in range(OC):
                    w_t = wp.tile([128, CC, 9, 128], F16, tag="w")
                    nc.sync.dma_start(out=w_t, in_=w_aps["v"][oc])
                    vs = vstp.tile([128, N], F32R, tag="vs")
                    for (y0, rr) in NT:
                        ps = cps.tile([128, rr * 48], F32, tag="conv")
                        first = True
                        for cc in range(CC):
                            for ky in range(3):
                                for kx in range(3):
                                    rhs = xpad_t[:, cc, y0 + ky:y0 + ky + rr, kx:kx + 48]
                                    nc.tensor.matmul(
                                        ps, w_t[:, cc, ky * 3 + kx, :], rhs,
                                        start=first, stop=(cc == CC - 1 and ky == 2 and kx == 2),
                                    )
                                    first = False
                        nc.scalar.activation(
                            out=vs[:, y0 * 48:(y0 + rr) * 48], in_=ps,
                            func=mybir.ActivationFunctionType.Identity,
                            bias=bias_t["v"][:, oc:oc + 1], scale=1.0,
                        )
                    for jc in range(JC):
                        tp = tps.tile([128, 128], F32R, tag="t")
                        nc.tensor.transpose(tp, vs[:, jc * 128:(jc + 1) * 128], ident)
                        # Act engine copies PSUM->SBUF with cast to bf16
                        nc.scalar.copy(out=vT[:, jc, oc * 128:(oc + 1) * 128], in_=tp)

        # ---------------- attention ----------------
        with tc.tile_pool(name="pp", bufs=2) as pp, \
             tc.tile_pool(name="trp", bufs=2) as trp, \
             tc.tile_pool(name="esb", bufs=2) as esb, \
             tc.tile_pool(name="sps", bufs=3, space="PSUM") as sps, \
             tc.tile_pool(name="aps", bufs=2, space="PSUM") as aps, \
             tc.tile_pool(name="rps", bufs=1, space="PSUM") as rps, \
             tc.tile_pool(name="cps2", bufs=1, space="PSUM") as cps2:
            p_tiles = {}

            def emit_qk(t):
                i0, iw = IT[t]
                p_t = pp.tile([128, JC, iw], BF16, tag="p", name=f"p_{t}")
                p_tiles[t] = p_t
                for jc in range(JC):
                    ps = sps.tile([128, iw], F32, tag="s", name=f"s_{t}_{jc}")
                    for ec in range(OC):
                        nc.tensor.matmul(
                            ps, k_res[:, ec, jc * 128:(jc + 1) * 128],
                            q_res[:, ec, i0:i0 + iw],
                            start=(ec == 0), stop=(ec == OC - 1),
                        )
                    nc.scalar.activation(
                        out=p_t[:, jc, :], in_=ps,
                        func=mybir.ActivationFunctionType.Exp,
                        bias=negC[:, 0:1], scale=1.0,
                    )

            def emit_post(t):
                i0, iw = IT[t]
                nsub = iw // 128
                p_t = p_tiles.pop(t)
                # rowsum tree on DVE: 18 = 8+8+2
                t8 = trp.tile([128, 8, iw], F32, tag="t8", name=f"t8_{t}")
                t4 = trp.tile([128, 4, iw], F32, tag="t4", name=f"t4_{t}")
                t2 = trp.tile([128, 2, iw], F32, tag="t2", name=f"t2_{t}")
                tx = trp.tile([128, 1, iw], F32, tag="tx", name=f"tx_{t}")
                t1 = trp.tile([128, iw], F32R, tag="t1", name=f"t1_{t}")
                with nc.allow_low_precision(reason="f32-accurate rowsum tree"):
                    nc.vector.tensor_tensor(t8, p_t[:, 0:8, :], p_t[:, 8:16, :],
                                            mybir.AluOpType.add)
                    nc.vector.tensor_tensor(tx, p_t[:, 16:17, :], p_t[:, 17:18, :],
                                            mybir.AluOpType.add)
                    nc.vector.tensor_tensor(t4, t8[:, 0:4, :], t8[:, 4:8, :],
                                            mybir.AluOpType.add)
                    nc.vector.tensor_tensor(t2, t4[:, 0:2, :], t4[:, 2:4, :],
                                            mybir.AluOpType.add)
                    nc.vector.tensor_tensor(t2[:, 0:1, :], t2[:, 0:1, :], t2[:, 1:2, :],
                                            mybir.AluOpType.add)
                    nc.vector.tensor_tensor(t1, t2[:, 0, :], tx[:, 0, :],
                                            mybir.AluOpType.add)
                rs = rps.tile([1, iw], F32, tag="rs", name=f"rs_{t}")
                nc.tensor.matmul(rs, ones_col, t1, start=True, stop=True)
                rs_sb = esb.tile([1, iw], F32R, tag="rsb", name=f"rsb_{t}")
                with nc.allow_low_precision(reason="copy of f32 psum"):
                    nc.vector.tensor_copy(out=rs_sb, in_=rs)

                for sub in range(nsub):
                    ic = i0 // 128 + sub
                    # broadcast rowsum slice onto partitions: [1,128]x[1,1]
                    rc_ps = cps2.tile([128, 2], F32, tag="rc", name=f"rc_{t}_{sub}")
                    nc.tensor.matmul(rc_ps, rs_sb[:, sub * 128:(sub + 1) * 128],
                                     ones_one, start=True, stop=True)
                    rcol = esb.tile([128, 2], F32, tag="rcol", name=f"rcol_{t}_{sub}")
                    nc.vector.reciprocal(out=rcol, in_=rc_ps)
                    # flipped AV: out^T[i, e] accumulating over j chunks
                    av = aps.tile([128, E], F32, tag="av", name=f"av_{t}_{sub}")
                    for jc in range(JC):
                        nc.tensor.matmul(
                            av, p_t[:, jc, sub * 128:(sub + 1) * 128],
                            vT[:, jc, :],
                            start=(jc == 0), stop=(jc == JC - 1),
                        )
                    o_bf = esb.tile([128, E], BF16, tag="obf", name=f"obf_{t}_{sub}")
                    nc.scalar.activation(
                        out=o_bf, in_=av,
                        func=mybir.ActivationFunctionType.Copy,
                        bias=0.0, scale=rcol[:, 0:1],
                    )
                    o_t = esb.tile([128, E], F32, tag="o", name=f"o_{t}_{sub}")
                    nc.vector.tensor_tensor(o_t, o_bf, vT[:, ic, :],
                                            mybir.AluOpType.add)
                    nc.sync.dma_start(out=out_ap[ic], in_=o_t)

            emit_qk(0)
            for t in range(1, len(IT)):
                emit_qk(t)
                emit_post(t - 1)
            emit_post(len(IT) - 1)

    nc.compile()
    return nc


def _prep_shared(Wq, bq, Wk, bk, Wv, bv):
    def wprep(Wm):
        A = Wm.reshape(OC, 128, CC, 128, 3, 3)
        Bm = A.transpose(0, 3, 2, 4, 5, 1)      # [oc, c, cc, ky, kx, o]
        return np.ascontiguousarray(
            Bm.reshape(OC, 128, CC, 9, 128), dtype=np.float16)

    def bprep(bm):
        return np.ascontiguousarray(bm.reshape(OC, 128).T, dtype=np.float32)

    return {
        "wq": wprep(Wq), "wk": wprep(Wk), "wv": wprep(Wv),
        "bq": bprep(bq), "bk": bprep(bk), "bv": bprep(bv),
    }


def kernel(feat, Wq, bq, Wk, bk, Wv, bv):
    feat = np.asarray(feat, dtype=np.float32)
    if "nc" not in _CACHE:
        _CACHE["nc"] = _build()
    nc = _CACHE["nc"]

    shared = _prep_shared(np.asarray(Wq, np.float32), np.asarray(bq, np.float32),
                          np.asarray(Wk, np.float32), np.asarray(bk, np.float32),
                          np.asarray(Wv, np.float32), np.asarray(bv, np.float32))

    in_maps = []
    for b in range(B):
        xpad = np.zeros((C, 50, 50), np.float16)
        xpad[:, 1:49, 1:49] = feat[b]
        xpad = np.ascontiguousarray(
            xpad.reshape(CC, 128, 2500).transpose(1, 0, 2)
        )
        in_maps.append({"xpad": xpad, **shared})

    r = bass_utils.run_bass_kernel_spmd(nc, in_maps, list(range(B)))
    out = np.stack(
        [np.ascontiguousarray(
            r.results[b]["out"].reshape(N, E).T).reshape(E, H, W)
         for b in range(B)], axis=0
    )
    return out


# revision 11
# speedup vs baseline: 1.1485x; 1.0073x over previous
"""ConvSA kernel for Trainium2 (8 NeuronCores, data-parallel over batch).

Computes, per batch element b (one per core):
    q/k/v = conv3x3(feat, W{q,k,v}) + b{q,k,v}        # 256 -> 512 ch, SAME pad
    att   = softmax_j(q^T k);  out = v @ att^T + v    # N = 48*48 = 2304

v2 strategy vs v1 (460us): fp16 operands for convs + QK (LDWEIGHTS at
1 cy/row makes matmuls stream-bound instead of weight-load-bound), bf16
for p=exp(s-C) and vT (bf16 keeps fp32 exponent range; fp16 would
underflow rows whose max logit is far below the global shift C).
Attention-value matmul is flipped to produce out^T tiles [i_part, e]:
p slices are the stationary operand, vT streams; the epilogue then
normalizes with a per-partition scalar (activation scale), adds the
residual v^T directly from vT (v_scr DRAM roundtrip eliminated), and
row-sums come from a DVE pairwise tree + one ones-matmul instead of 18
PE matmuls per tile. Output is written transposed [N, E]; the host
untransposes.
"""
import numpy as np
from contextlib import ExitStack

import concourse.bass as bass
import concourse.tile as tile
from concourse import bacc, bass_utils, mybir
from concourse.masks import make_identity

F32 = mybir.dt.float32
F32R = mybir.dt.float32r
F16 = mybir.dt.float16
BF16 = mybir.dt.bfloat16

B, C, H, W = 8, 256, 48, 48
E = 512
N = H * W            # 2304
CC = C // 128        # 2 c-chunks
OC = E // 128        # 4 o-chunks / e-chunks
JC = N // 128        # 18 j-chunks
NT = [(0, 10), (10, 10), (20, 10), (30, 10), (40, 8)]     # conv row tiles
IT = [(0, 512), (512, 512), (1024, 512), (1536, 512), (2048, 256)]  # i tiles

_CACHE = {}


def _build():
    nc = bacc.Bacc("TRN2", target_bir_lowering=False, debug=False, num_devices=B)

    xp_ap = nc.dram_tensor("xpad", [128, CC, 2500], F16, kind="ExternalInput").ap()
    w_aps = {
        cn: nc.dram_tensor(f"w{cn}", [OC, 128, CC, 9, 128], F16, kind="ExternalInput").ap()
        for cn in "qkv"
    }
    b_aps = {
        cn: nc.dram_tensor(f"b{cn}", [128, OC], F32, kind="ExternalInput").ap()
        for cn in "qkv"
    }
    # transposed output: [i-chunk, 128 i, E]
    out_ap = nc.dram_tensor("out", [JC, 128, E], F32, kind="ExternalOutput").ap()

    with tile.TileContext(nc) as tc, ExitStack() as ctx:
        res = ctx.enter_context(tc.tile_pool(name="res", bufs=1))
        k_res = res.tile([128, OC, N], F16, tag="k")
        q_res = res.tile([128, OC, N], F16, tag="q")
        vT = res.tile([128, JC, E], BF16, tag="vT")
        bias_t = {cn: res.tile([128, OC], F32, tag=f"b{cn}", name=f"bias_{cn}")
                  for cn in "qkv"}
        ones_col = res.tile([128, 1], F32R, tag="oc")
        ones_row = res.tile([1, 128], F32R, tag="or")
        ones_one = res.tile([1, 2], F32R, tag="o1")
        negC = res.tile([128, 1], F32, tag="negc")
        ident = res.tile([128, 128], F32R, tag="id")

        # ---------------- conv phase ----------------
        with tc.tile_pool(name="xw", bufs=1) as xwp, \
             tc.tile_pool(name="w", bufs=3) as wp, \
             tc.tile_pool(name="vst", bufs=2) as vstp, \
             tc.tile_pool(name="cps", bufs=2, space="PSUM") as cps:
            xpad_t = xwp.tile([128, CC, 50, 50], F16, tag="x")
            # DMA order tuned for time-to-first-matmul: xpad cc0 rows 0-24,
            # then the first K-conv weight tile, then the rest of the image.
            nc.sync.dma_start(
                out=xpad_t[:, 0, 0:25].rearrange("p h w -> p (h w)"),
                in_=xp_ap[:, 0, 0:1250],
            )
            w_k0 = wp.tile([128, CC, 9, 128], F16, tag="w", name="w_k0")
            nc.sync.dma_start(out=w_k0, in_=w_aps["k"][0])
            nc.sync.dma_start(
                out=xpad_t[:, 0, 25:50].rearrange("p h w -> p (h w)"),
                in_=xp_ap[:, 0, 1250:2500],
            )
            nc.sync.dma_start(
                out=xpad_t[:, 1].rearrange("p h w -> p (h w)"),
                in_=xp_ap[:, 1],
            )
            ident_raw = xwp.tile([128, 128], F32, tag="idr")
            make_identity(nc, ident_raw)
            nc.vector.tensor_copy(out=ident, in_=ident_raw)
            ones_raw = xwp.tile([128, 1], F32, tag="onr")
            nc.vector.memset(ones_raw, 1.0)
            nc.vector.tensor_copy(out=ones_col, in_=ones_raw)
            ones_raw2 = xwp.tile([1, 128], F32, tag="onr2")
            nc.vector.memset(ones_raw2, 1.0)
            nc.vector.tensor_copy(out=ones_row, in_=ones_raw2)
            nc.vector.tensor_copy(out=ones_one, in_=ones_raw2[:, 0:2])
            for cn in "qkv":
                nc.sync.dma_start(out=bias_t[cn], in_=b_aps[cn])

            def conv(cn, sink, out_dt, w_pre=None):
                for oc in range(OC):
                    if oc == 0 and w_pre is not None:
                        w_t = w_pre
                    else:
                        w_t = wp.tile([128, CC, 9, 128], F16, tag="w")
                        nc.sync.dma_start(out=w_t, in_=w_aps[cn][oc])
                    for (y0, rr) in NT:
                        ps = cps.tile([128, rr * 48], F32, tag="conv")
                        first = True
                        for cc in range(CC):
                            for ky in range(3):
                                for kx in range(3):
                                    rhs = xpad_t[:, cc, y0 + ky:y0 + ky + rr, kx:kx + 48]
                                    nc.tensor.matmul(
                                        ps, w_t[:, cc, ky * 3 + kx, :], rhs,
                                        start=first, stop=(cc == CC - 1 and ky == 2 and kx == 2),
                                    )
                                    first = False
                        sink(cn, oc, y0, rr, ps)

            def to_res(dst):
                def sink(cn, oc, y0, rr, ps):
                    nc.scalar.activation(
                        out=dst[:, oc, y0 * 48:(y0 + rr) * 48], in_=ps,
                        func=mybir.ActivationFunctionType.Identity,
                        bias=bias_t[cn][:, oc:oc + 1], scale=1.0,
                    )
                return sink

            conv("k", to_res(k_res), F16, w_pre=w_k0)
            conv("q", to_res(q_res), F16)

            # ---- global shift constant C (hidden under V conv) ----
            # C = max over i in [0,256) x j in [0,1024) of s -- any constant
            # with  rowmax-80 <= C <= globalmax+88  keeps exp() in fp32/bf16
            # range, and softmax is shift-invariant so the result is exact.
            with tc.tile_pool(name="mps", bufs=1, space="PSUM") as mps, \
                 tc.tile_pool(name="msb", bufs=1) as msb, \
                 tc.tile_pool(name="nps", bufs=1, space="PSUM") as nps:
                mini = mps.tile([128, 4, 256], F32)
                for jc in range(4):
                    for ec in range(OC):
                        nc.tensor.matmul(
                            mini[:, jc, :], k_res[:, ec, jc * 128:(jc + 1) * 128],
                            q_res[:, ec, 0:256], start=(ec == 0), stop=(ec == OC - 1),
                        )
                m1 = msb.tile([128, 1], F32R, tag="m1")
                nc.vector.reduce_max(out=m1, in_=mini, axis=mybir.AxisListType.XY)
                tpm = nps.tile([1, 128], F32R, tag="tp")
                nc.tensor.transpose(tpm, m1, ident)
                cneg = msb.tile([1, 2], F32R, tag="cn")
                nc.vector.reduce_max(out=cneg[:, 0:1], in_=tpm,
                                     axis=mybir.AxisListType.X, negate=True)
                nc.vector.tensor_copy(out=cneg[:, 1:2], in_=cneg[:, 0:1])
                ncps = nps.tile([128, 2], F32, tag="ncps")
                nc.tensor.matmul(ncps, ones_row, cneg, start=True, stop=True)
                nc.vector.tensor_copy(out=negC, in_=ncps[:, 0:1])

            # v conv: per o-chunk, transpose into vT (bf16); residual v^T
            # is read straight out of vT later, no DRAM scratch needed.
            with tc.tile_pool(name="tps", bufs=2, space="PSUM") as tps:
                for oc in range(OC):
                    w_t = wp.tile([128, CC, 9, 128], F16, tag="w")
                    nc.sync.dma_start(out=w_t, in_=w_aps["v"][oc])
                    vs = vstp.tile([128, N], F32R, tag="vs")
                    for (y0, rr) in NT:
                        ps = cps.tile([128, rr * 48], F32, tag="conv")
                        first = True
                        for cc in range(CC):
                            for ky in range(3):
                                for kx in range(3):
                                    rhs = xpad_t[:, cc, y0 + ky:y0 + ky + rr, kx:kx + 48]
                                    nc.tensor.matmul(
                                        ps, w_t[:, cc, ky * 3 + kx, :], rhs,
                                        start=first, stop=(cc == CC - 1 and ky == 2 and kx == 2),
                                    )
                                    first = False
                        nc.scalar.activation(
                            out=vs[:, y0 * 48:(y0 + rr) * 48], in_=ps,
                            func=mybir.ActivationFunctionType.Identity,
                            bias=bias_t["v"][:, oc:oc + 1], scale=1.0,
                        )
                    for jc in range(JC):
                        tp = tps.tile([128, 128], F32R, tag="t")
                        nc.tensor.transpose(tp, vs[:, jc * 128:(jc + 1) * 128], ident)
                        # Act engine copies PSUM->SBUF with cast to bf16
                        nc.scalar.copy(out=vT[:, jc, oc * 128:(oc + 1) * 128], in_=tp)

        # ---------------- attention ----------------
        with tc.tile_pool(name="pp", bufs=2) as pp, \
             tc.tile_pool(name="trp", bufs=2) as trp, \
             tc.tile_pool(name="esb", bufs=2) as esb, \
             tc.tile_pool(name="sps", bufs=3, space="PSUM") as sps, \
             tc.tile_pool(name="aps", bufs=3, space="PSUM") as aps, \
             tc.tile_pool(name="rps", bufs=1, space="PSUM") as rps, \
             tc.tile_pool(name="cps2", bufs=1, space="PSUM") as cps2:
            p_tiles = {}

            def emit_qk(t):
                i0, iw = IT[t]
                p_t = pp.tile([128, JC, iw], BF16, tag="p", name=f"p_{t}")
                p_tiles[t] = p_t
                for jc in range(JC):
                    ps = sps.tile([128, iw], F32, tag="s", name=f"s_{t}_{jc}")
                    for ec in range(OC):
                        nc.tensor.matmul(
                            ps, k_res[:, ec, jc * 128:(jc + 1) * 128],
                            q_res[:, ec, i0:i0 + iw],
                            start=(ec == 0), stop=(ec == OC - 1),
                        )
                    nc.scalar.activation(
                        out=p_t[:, jc, :], in_=ps,
                        func=mybir.ActivationFunctionType.Exp,
                        bias=negC[:, 0:1], scale=1.0,
                    )

            def emit_post(t):
                i0, iw = IT[t]
                nsub = iw // 128
                p_t = p_tiles.pop(t)
                # rowsum tree on DVE: 18 = 8+8+2
                t8 = trp.tile([128, 8, iw], F32, tag="t8", name=f"t8_{t}")
                t4 = trp.tile([128, 4, iw], F32, tag="t4", name=f"t4_{t}")
                t2 = trp.tile([128, 2, iw], F32, tag="t2", name=f"t2_{t}")
                tx = trp.tile([128, 1, iw], F32, tag="tx", name=f"tx_{t}")
                t1 = trp.tile([128, iw], F32R, tag="t1", name=f"t1_{t}")
                with nc.allow_low_precision(reason="f32-accurate rowsum tree"):
                    nc.vector.tensor_tensor(t8, p_t[:, 0:8, :], p_t[:, 8:16, :],
                                            mybir.AluOpType.add)
                    nc.vector.tensor_tensor(tx, p_t[:, 16:17, :], p_t[:, 17:18, :],
                                            mybir.AluOpType.add)
                    nc.vector.tensor_tensor(t4, t8[:, 0:4, :], t8[:, 4:8, :],
                                            mybir.AluOpType.add)
                    nc.vector.tensor_tensor(t2, t4[:, 0:2, :], t4[:, 2:4, :],
                                            mybir.AluOpType.add)
                    nc.vector.tensor_tensor(t2[:, 0:1, :], t2[:, 0:1, :], t2[:, 1:2, :],
                                            mybir.AluOpType.add)
                    nc.vector.tensor_tensor(t1, t2[:, 0, :], tx[:, 0, :],
                                            mybir.AluOpType.add)
                # PE order: AV groups 0..nsub-2, then the short rowsum chain
                # (whose DVE inputs finished long ago), then the last AV
                # group; Act scales + epilogue drain in between.
                avs, rcols = {}, {}

                def emit_av(sub):
                    av = aps.tile([128, E], F32, tag="av", name=f"av_{t}_{sub}")
                    avs[sub] = av
                    for jc in range(JC):
                        nc.tensor.matmul(
                            av, p_t[:, jc, sub * 128:(sub + 1) * 128],
                            vT[:, jc, :],
                            start=(jc == 0), stop=(jc == JC - 1),
                        )

                def emit_scale(sub):
                    ic = i0 // 128 + sub
                    o_bf = esb.tile([128, E], BF16, tag="obf", name=f"obf_{t}_{sub}")
                    nc.scalar.activation(
                        out=o_bf, in_=avs[sub],
                        func=mybir.ActivationFunctionType.Copy,
                        bias=0.0, scale=rcols[sub][:, 0:1],
                    )
                    o_t = esb.tile([128, E], F32, tag="o", name=f"o_{t}_{sub}")
                    nc.vector.tensor_tensor(o_t, o_bf, vT[:, ic, :],
                                            mybir.AluOpType.add)
                    nc.sync.dma_start(out=out_ap[ic], in_=o_t)

                for sub in range(nsub - 1):
                    emit_av(sub)

                rs = rps.tile([1, iw], F32, tag="rs", name=f"rs_{t}")
                nc.tensor.matmul(rs, ones_col, t1, start=True, stop=True)
                rs_sb = esb.tile([1, iw], F32R, tag="rsb", name=f"rsb_{t}")
                with nc.allow_low_precision(reason="copy of f32 psum"):
                    nc.vector.tensor_copy(out=rs_sb, in_=rs)
                for sub in range(nsub):
                    # broadcast rowsum slice onto partitions: [1,128]x[1,2]
                    rc_ps = cps2.tile([128, 2], F32, tag="rc", name=f"rc_{t}_{sub}")
                    nc.tensor.matmul(rc_ps, rs_sb[:, sub * 128:(sub + 1) * 128],
                                     ones_one, start=True, stop=True)
                    rcol = esb.tile([128, 2], F32, tag="rcol", name=f"rcol_{t}_{sub}")
                    rcols[sub] = rcol
                    nc.vector.reciprocal(out=rcol, in_=rc_ps)
                for sub in range(nsub - 1):
                    emit_scale(sub)
                emit_av(nsub - 1)
                emit_scale(nsub - 1)

            emit_qk(0)
            for t in range(1, len(IT)):
                emit_qk(t)
                emit_post(t - 1)
            emit_post(len(IT) - 1)

    nc.compile()
    return nc


def _prep_shared(Wq, bq, Wk, bk, Wv, bv):
    def wprep(Wm):
        A = Wm.reshape(OC, 128, CC, 128, 3, 3)
        Bm = A.transpose(0, 3, 2, 4, 5, 1)      # [oc, c, cc, ky, kx, o]
        return np.ascontiguousarray(
            Bm.reshape(OC, 128, CC, 9, 128), dtype=np.float16)

    def bprep(bm):
        return np.ascontiguousarray(bm.reshape(OC, 128).T, dtype=np.float32)

    return {
        "wq": wprep(Wq), "wk": wprep(Wk), "wv": wprep(Wv),
        "bq": bprep(bq), "bk": bprep(bk), "bv": bprep(bv),
    }


def kernel(feat, Wq, bq, Wk, bk, Wv, bv):
    feat = np.asarray(feat, dtype=np.float32)
    if "nc" not in _CACHE:
        _CACHE["nc"] = _build()
    nc = _CACHE["nc"]

    shared = _prep_shared(np.asarray(Wq, np.float32), np.asarray(bq, np.float32),
                          np.asarray(Wk, np.float32), np.asarray(bk, np.float32),
                          np.asarray(Wv, np.float32), np.asarray(bv, np.float32))

    in_maps = []
    for b in range(B):
        xpad = np.zeros((C, 50, 50), np.float16)
        xpad[:, 1:49, 1:49] = feat[b]
        xpad = np.ascontiguousarray(
            xpad.reshape(CC, 128, 2500).transpose(1, 0, 2)
        )
        in_maps.append({"xpad": xpad, **shared})

    r = bass_utils.run_bass_kernel_spmd(nc, in_maps, list(range(B)))
    out = np.stack(
        [np.ascontiguousarray(
            r.results[b]["out"].reshape(N, E).T).reshape(E, H, W)
         for b in range(B)], axis=0
    )
    return out


# revision 19
# speedup vs baseline: 1.2193x; 1.0617x over previous
"""ConvSA kernel for Trainium2 (8 NeuronCores, data-parallel over batch).

Computes, per batch element b (one per core):
    q/k/v = conv3x3(feat, W{q,k,v}) + b{q,k,v}        # 256 -> 512 ch, SAME pad
    att   = softmax_j(q^T k);  out = v @ att^T + v    # N = 48*48 = 2304

v2 strategy vs v1 (460us): fp16 operands for convs + QK (LDWEIGHTS at
1 cy/row makes matmuls stream-bound instead of weight-load-bound), bf16
for p=exp(s-C) and vT (bf16 keeps fp32 exponent range; fp16 would
underflow rows whose max logit is far below the global shift C).
Attention-value matmul is flipped to produce out^T tiles [i_part, e]:
p slices are the stationary operand, vT streams; the epilogue then
normalizes with a per-partition scalar (activation scale), adds the
residual v^T directly from vT (v_scr DRAM roundtrip eliminated), and
row-sums come from a DVE pairwise tree + one ones-matmul instead of 18
PE matmuls per tile. Output is written transposed [N, E]; the host
untransposes.
"""
import numpy as np
from contextlib import ExitStack

import concourse.bass as bass
import concourse.tile as tile
from concourse import bacc, bass_utils, mybir
from concourse.masks import make_identity

F32 = mybir.dt.float32
F32R = mybir.dt.float32r
F16 = mybir.dt.float16
BF16 = mybir.dt.bfloat16

B, C, H, W = 8, 256, 48, 48
E = 512
N = H * W            # 2304
CC = C // 128        # 2 c-chunks
OC = E // 128        # 4 o-chunks / e-chunks
JC = N // 128        # 18 j-chunks
NT = [(0, 10), (10, 10), (20, 10), (30, 10), (40, 8)]     # conv row tiles
IT = [(0, 512), (512, 512), (1024, 512), (1536, 512), (2048, 256)]  # i tiles

_CACHE = {}


def _build():
    nc = bacc.Bacc("TRN2", target_bir_lowering=False, debug=False, num_devices=B)

    xp_ap = nc.dram_tensor("xpad", [128, CC, 2500], F16, kind="ExternalInput").ap()
    # K/Q conv weights in 1-D Winograd F(2,3) form: [oc, c, m, ky, cc, e]
    wg_aps = {
        cn: nc.dram_tensor(f"wg{cn}", [OC, 128, 4, 3, CC, 128], F16,
                           kind="ExternalInput").ap()
        for cn in "qk"
    }
    br_aps = {
        cn: nc.dram_tensor(f"br{cn}", [1, E], F16, kind="ExternalInput").ap()
        for cn in "qk"
    }
    w_aps = {
        cn: nc.dram_tensor(f"w{cn}", [OC, 128, CC, 9, 128], F16, kind="ExternalInput").ap()
        for cn in "v"
    }
    b_aps = {
        cn: nc.dram_tensor(f"b{cn}", [128, OC], F32, kind="ExternalInput").ap()
        for cn in "v"
    }
    # transposed output: [i-chunk, 128 i, E]
    out_ap = nc.dram_tensor("out", [JC, 128, E], F32, kind="ExternalOutput").ap()

    with tile.TileContext(nc) as tc, ExitStack() as ctx:
        res = ctx.enter_context(tc.tile_pool(name="res", bufs=1))
        k_res = res.tile([128, OC, N], F16, tag="k")
        q_res = res.tile([128, OC, N], F16, tag="q")
        vT = res.tile([128, JC, E], BF16, tag="vT")
        bias_t = {cn: res.tile([128, OC], F32, tag=f"b{cn}", name=f"bias_{cn}")
                  for cn in "v"}
        br_t = {cn: res.tile([1, E], F16, tag=f"br{cn}", name=f"biasr_{cn}")
                for cn in "qk"}
        ones384 = res.tile([1, 512], F16, tag="on384")
        ones_col = res.tile([128, 1], F32R, tag="oc")
        ones_row = res.tile([1, 128], F32R, tag="or")
        ones_one = res.tile([1, 2], F32R, tag="o1")
        negC = res.tile([128, 1], F32, tag="negc")
        ident = res.tile([128, 128], F32R, tag="id")

        # ---------------- conv phase ----------------
        # K/Q: 1-D Winograd F(2,3) along x (1.5x fewer PE MACs).
        # For each output pair (y, 2t / 2t+1):
        #   D0 = x0-x2, D1 = x1+x2, D2 = x2-x1, D3 = x1-x3  (x_k = xpad col 2t+k)
        #   Y_m = sum_{ky,c} G_m(w) * D_m ;  y0 = Y0+Y1+Y2, y1 = Y1-Y2-Y3
        # with G rows {w0, (w0+w1+w2)/2, (w0-w1+w2)/2, w2}; conv bias is
        # injected into the Y1 accumulation via a [1,e]x[1,384] matmul.
        xpad_t = res.tile([128, CC, 50, 50], F16, tag="x")
        with tc.tile_pool(name="xw", bufs=1) as xwp, \
             tc.tile_pool(name="wg", bufs=3) as wgp, \
             tc.tile_pool(name="wtmp", bufs=3) as wtp, \
             tc.tile_pool(name="wps", bufs=2, space="PSUM") as wps:
            xt = xwp.tile([128, 4, CC, 50, 24], F16, tag="xt")

            def emit_xt(cc):
                xr = xpad_t[:, cc].rearrange("p h (x two) -> p h x two", two=2)
                x0 = xr[:, :, 0:24, 0]
                x1 = xr[:, :, 0:24, 1]
                x2 = xr[:, :, 1:25, 0]
                x3 = xr[:, :, 1:25, 1]
                with nc.allow_low_precision(reason="fp16 winograd input tf"):
                    nc.vector.tensor_tensor(xt[:, 0, cc], x0, x2,
                                            mybir.AluOpType.subtract)
                    nc.vector.tensor_tensor(xt[:, 1, cc], x1, x2,
                                            mybir.AluOpType.add)
                    nc.vector.tensor_tensor(xt[:, 2, cc], x2, x1,
                                            mybir.AluOpType.subtract)
                    nc.vector.tensor_tensor(xt[:, 3, cc], x1, x3,
                                            mybir.AluOpType.subtract)

            # head DMA order: xpad cc0, first m-chunk of K weights, rest
            nc.sync.dma_start(
                out=xpad_t[:, 0].rearrange("p h w -> p (h w)"), in_=xp_ap[:, 0])
            wg_k0 = wgp.tile([128, 4, 3, CC, 128], F16, tag="wg", name="wg_k0")
            nc.sync.dma_start(out=wg_k0[:, 0], in_=wg_aps["k"][0][:, 0])
            emit_xt(0)
            nc.sync.dma_start(out=wg_k0[:, 1:4], in_=wg_aps["k"][0][:, 1:4])
            nc.sync.dma_start(
                out=xpad_t[:, 1].rearrange("p h w -> p (h w)"), in_=xp_ap[:, 1])
            emit_xt(1)
            for cn in "kq":
                nc.sync.dma_start(out=br_t[cn], in_=br_aps[cn])
            nc.sync.dma_start(out=bias_t["v"], in_=b_aps["v"])
            ident_raw = xwp.tile([128, 128], F32, tag="idr")
            make_identity(nc, ident_raw)
            nc.vector.tensor_copy(out=ident, in_=ident_raw)
            ones_raw = xwp.tile([128, 1], F32, tag="onr")
            nc.vector.memset(ones_raw, 1.0)
            nc.vector.tensor_copy(out=ones_col, in_=ones_raw)
            ones_raw2 = xwp.tile([1, 128], F32, tag="onr2")
            nc.vector.memset(ones_raw2, 1.0)
            nc.vector.tensor_copy(out=ones_row, in_=ones_raw2)
            nc.vector.tensor_copy(out=ones_one, in_=ones_raw2[:, 0:2])
            nc.vector.memset(ones384, 1.0)

            def wconv(cn, dst, wg_pre=None):
                dstr = dst.rearrange("p o (y x two) -> p o y x two",
                                     y=48, two=2)
                for oc in range(OC):
                    if oc == 0 and wg_pre is not None:
                        wt = wg_pre
                    else:
                        wt = wgp.tile([128, 4, 3, CC, 128], F16, tag="wg")
                        nc.sync.dma_start(out=wt, in_=wg_aps[cn][oc])
                    for th in range(3):
                        mt = [wps.tile([128, 512], F32, tag=f"m{m}",
                                       name=f"m{m}_{cn}_{oc}_{th}")
                              for m in range(4)]
                        for m in range(4):
                            for ky in range(3):
                                for cc in range(CC):
                                    nc.tensor.matmul(
                                        mt[m][:, 0:384],
                                        wt[:, m, ky, cc, :],
                                        xt[:, m, cc, ky:ky + 48,
                                           th * 8:th * 8 + 8],
                                        start=(ky == 0 and cc == 0),
                                        stop=(ky == 2 and cc == 1 and m != 1),
                                    )
                            if m == 1:
                                nc.tensor.matmul(
                                    mt[1][:, 0:384],
                                    br_t[cn][:, oc * 128:(oc + 1) * 128],
                                    ones384[:, 0:384],
                                    start=False, stop=True,
                                )
                        mv = [mt[m][:, 0:384].rearrange(
                            "p (y x) -> p y x", y=48) for m in range(4)]
                        # DVE reads at most one PSUM operand per op, so the
                        # Act engine first stages m1/m2 into SBUF.
                        m1s = wtp.tile([128, 384], F32, tag="m1s",
                                       name=f"m1s_{cn}_{oc}_{th}")
                        nc.scalar.copy(out=m1s, in_=mt[1][:, 0:384])
                        m1sv = m1s.rearrange("p (y x) -> p y x", y=48)
                        m2s = wtp.tile([128, 384], F32, tag="m2s",
                                       name=f"m2s_{cn}_{oc}_{th}")
                        nc.scalar.copy(out=m2s, in_=mt[2][:, 0:384])
                        m2sv = m2s.rearrange("p (y x) -> p y x", y=48)
                        t01 = wtp.tile([128, 384], F32, tag="t01",
                                       name=f"t01_{cn}_{oc}_{th}")
                        t01v = t01.rearrange("p (y x) -> p y x", y=48)
                        t23 = wtp.tile([128, 384], F32, tag="t23",
                                       name=f"t23_{cn}_{oc}_{th}")
                        t23v = t23.rearrange("p (y x) -> p y x", y=48)
                        d0 = dstr[:, oc, :, th * 8:th * 8 + 8, 0]
                        d1 = dstr[:, oc, :, th * 8:th * 8 + 8, 1]
                        with nc.allow_low_precision(reason="winograd out tf"):
                            nc.vector.tensor_tensor(t01v, mv[0], m1sv,
                                                    mybir.AluOpType.add)
                            nc.vector.tensor_tensor(t23v, m2sv, mv[3],
                                                    mybir.AluOpType.add)
                            nc.vector.tensor_tensor(d0, t01v, mv[2],
                                                    mybir.AluOpType.add)
                            nc.vector.tensor_tensor(d1, mv[1], t23v,
                                                    mybir.AluOpType.subtract)

            wconv("k", k_res, wg_pre=wg_k0)
            wconv("q", q_res)

        # ---- global shift constant C ----
        # C = max over i in [0,256) x j in [0,512) of s -- any constant
        # with  rowmax-80 <= C <= globalmax+88  keeps exp() in fp32/bf16
        # range, and softmax is shift-invariant so the result is exact.
        with tc.tile_pool(name="mps", bufs=1, space="PSUM") as mps, \
             tc.tile_pool(name="msb", bufs=1) as msb, \
             tc.tile_pool(name="nps", bufs=1, space="PSUM") as nps:
            mini = mps.tile([128, 4, 256], F32)
            for jc in range(4):
                for ec in range(OC):
                    nc.tensor.matmul(
                        mini[:, jc, :], k_res[:, ec, jc * 128:(jc + 1) * 128],
                        q_res[:, ec, 0:256], start=(ec == 0), stop=(ec == OC - 1),
                    )
            m1 = msb.tile([128, 1], F32R, tag="m1")
            nc.vector.reduce_max(out=m1, in_=mini, axis=mybir.AxisListType.XY)
            tpm = nps.tile([1, 128], F32R, tag="tp")
            nc.tensor.transpose(tpm, m1, ident)
            cneg = msb.tile([1, 2], F32R, tag="cn")
            nc.vector.reduce_max(out=cneg[:, 0:1], in_=tpm,
                                 axis=mybir.AxisListType.X, negate=True)
            nc.vector.tensor_copy(out=cneg[:, 1:2], in_=cneg[:, 0:1])
            ncps = nps.tile([128, 2], F32, tag="ncps")
            nc.tensor.matmul(ncps, ones_row, cneg, start=True, stop=True)
            nc.vector.tensor_copy(out=negC, in_=ncps[:, 0:1])

        # v conv (direct): per o-chunk, transpose into vT (bf16); residual
        # v^T is read straight out of vT later, no DRAM scratch needed.
        with tc.tile_pool(name="w", bufs=3) as wp, \
             tc.tile_pool(name="vst", bufs=2) as vstp, \
             tc.tile_pool(name="cps", bufs=2, space="PSUM") as cps, \
             tc.tile_pool(name="tps", bufs=2, space="PSUM") as tps:
            for oc in range(OC):
                w_t = wp.tile([128, CC, 9, 128], F16, tag="w")
                nc.sync.dma_start(out=w_t, in_=w_aps["v"][oc])
                vs = vstp.tile([128, N], F32R, tag="vs")
                for (y0, rr) in NT:
                    ps = cps.tile([128, rr * 48], F32, tag="conv")
                    first = True
                    for cc in range(CC):
                        for ky in range(3):
                            for kx in range(3):
                                rhs = xpad_t[:, cc, y0 + ky:y0 + ky + rr, kx:kx + 48]
                                nc.tensor.matmul(
                                    ps, w_t[:, cc, ky * 3 + kx, :], rhs,
                                    start=first, stop=(cc == CC - 1 and ky == 2 and kx == 2),
                                )
                                first = False
                    nc.scalar.activation(
                        out=vs[:, y0 * 48:(y0 + rr) * 48], in_=ps,
                        func=mybir.ActivationFunctionType.Identity,
                        bias=bias_t["v"][:, oc:oc + 1], scale=1.0,
                    )
                for jc in range(JC):
                    tp = tps.tile([128, 128], F32R, tag="t")
                    nc.tensor.transpose(tp, vs[:, jc * 128:(jc + 1) * 128], ident)
                    # Act engine copies PSUM->SBUF with cast to bf16
                    nc.scalar.copy(out=vT[:, jc, oc * 128:(oc + 1) * 128], in_=tp)

        # ---------------- attention ----------------
        with tc.tile_pool(name="pp", bufs=2) as pp, \
             tc.tile_pool(name="trp", bufs=2) as trp, \
             tc.tile_pool(name="esb", bufs=2) as esb, \
             tc.tile_pool(name="sps", bufs=3, space="PSUM") as sps, \
             tc.tile_pool(name="aps", bufs=3, space="PSUM") as aps, \
             tc.tile_pool(name="rps", bufs=1, space="PSUM") as rps, \
             tc.tile_pool(name="cps2", bufs=1, space="PSUM") as cps2:
            p_tiles = {}

            def emit_qk(t):
                i0, iw = IT[t]
                p_t = pp.tile([128, JC, iw], BF16, tag="p", name=f"p_{t}")
                p_tiles[t] = p_t
                for jc in range(JC):
                    ps = sps.tile([128, iw], F32, tag="s", name=f"s_{t}_{jc}")
                    for ec in range(OC):
                        nc.tensor.matmul(
                            ps, k_res[:, ec, jc * 128:(jc + 1) * 128],
                            q_res[:, ec, i0:i0 + iw],
                            start=(ec == 0), stop=(ec == OC - 1),
                        )
                    nc.scalar.activation(
                        out=p_t[:, jc, :], in_=ps,
                        func=mybir.ActivationFunctionType.Exp,
                        bias=negC[:, 0:1], scale=1.0,
                    )

            def emit_post(t):
                i0, iw = IT[t]
                nsub = iw // 128
                p_t = p_tiles.pop(t)
                # rowsum tree on DVE: 18 = 8+8+2
                t8 = trp.tile([128, 8, iw], F32, tag="t8", name=f"t8_{t}")
                t4 = trp.tile([128, 4, iw], F32, tag="t4", name=f"t4_{t}")
                t2 = trp.tile([128, 2, iw], F32, tag="t2", name=f"t2_{t}")
                tx = trp.tile([128, 1, iw], F32, tag="tx", name=f"tx_{t}")
                t1 = trp.tile([128, iw], F32R, tag="t1", name=f"t1_{t}")
                with nc.allow_low_precision(reason="f32-accurate rowsum tree"):
                    nc.vector.tensor_tensor(t8, p_t[:, 0:8, :], p_t[:, 8:16, :],
                                            mybir.AluOpType.add)
                    nc.vector.tensor_tensor(tx, p_t[:, 16:17, :], p_t[:, 17:18, :],
                                            mybir.AluOpType.add)
                    nc.vector.tensor_tensor(t4, t8[:, 0:4, :], t8[:, 4:8, :],
                                            mybir.AluOpType.add)
                    nc.vector.tensor_tensor(t2, t4[:, 0:2, :], t4[:, 2:4, :],
                                            mybir.AluOpType.add)
                    nc.vector.tensor_tensor(t2[:, 0:1, :], t2[:, 0:1, :], t2[:, 1:2, :],
                                            mybir.AluOpType.add)
                    nc.vector.tensor_tensor(t1, t2[:, 0, :], tx[:, 0, :],
                                            mybir.AluOpType.add)
                # PE order: AV groups 0..nsub-2, then the short rowsum chain
                # (whose DVE inputs finished long ago), then the last AV
                # group; Act scales + epilogue drain in between.
                avs, rcols = {}, {}

                def emit_av(sub):
                    av = aps.tile([128, E], F32, tag="av", name=f"av_{t}_{sub}")
                    avs[sub] = av
                    for jc in range(JC):
                        nc.tensor.matmul(
                            av, p_t[:, jc, sub * 128:(sub + 1) * 128],
                            vT[:, jc, :],
                            start=(jc == 0), stop=(jc == JC - 1),
                        )

                def emit_scale(sub):
                    ic = i0 // 128 + sub
                    o_bf = esb.tile([128, E], BF16, tag="obf", name=f"obf_{t}_{sub}")
                    nc.scalar.activation(
                        out=o_bf, in_=avs[sub],
                        func=mybir.ActivationFunctionType.Copy,
                        bias=0.0, scale=rcols[sub][:, 0:1],
                    )
                    o_t = esb.tile([128, E], F32, tag="o", name=f"o_{t}_{sub}")
                    nc.vector.tensor_tensor(o_t, o_bf, vT[:, ic, :],
                                            mybir.AluOpType.add)
                    nc.sync.dma_start(out=out_ap[ic], in_=o_t)

                for sub in range(nsub - 1):
                    emit_av(sub)

                rs = rps.tile([1, iw], F32, tag="rs", name=f"rs_{t}")
                nc.tensor.matmul(rs, ones_col, t1, start=True, stop=True)
                rs_sb = esb.tile([1, iw], F32R, tag="rsb", name=f"rsb_{t}")
                with nc.allow_low_precision(reason="copy of f32 psum"):
                    nc.vector.tensor_copy(out=rs_sb, in_=rs)
                for sub in range(nsub):
                    # broadcast rowsum slice onto partitions: [1,128]x[1,2]
                    rc_ps = cps2.tile([128, 2], F32, tag="rc", name=f"rc_{t}_{sub}")
                    nc.tensor.matmul(rc_ps, rs_sb[:, sub * 128:(sub + 1) * 128],
                                     ones_one, start=True, stop=True)
                    rcol = esb.tile([128, 2], F32, tag="rcol", name=f"rcol_{t}_{sub}")
                    rcols[sub] = rcol
                    nc.vector.reciprocal(out=rcol, in_=rc_ps)
                for sub in range(nsub - 1):
                    emit_scale(sub)
                emit_av(nsub - 1)
                emit_scale(nsub - 1)

            emit_qk(0)
            for t in range(1, len(IT)):
                emit_qk(t)
                emit_post(t - 1)
            emit_post(len(IT) - 1)

    nc.compile()
    return nc


def _prep_shared(Wq, bq, Wk, bk, Wv, bv):
    def wprep(Wm):
        A = Wm.reshape(OC, 128, CC, 128, 3, 3)
        Bm = A.transpose(0, 3, 2, 4, 5, 1)      # [oc, c, cc, ky, kx, o]
        return np.ascontiguousarray(
            Bm.reshape(OC, 128, CC, 9, 128), dtype=np.float16)

    def wprep_wino(Wm):
        # [oc, e, cc, c, ky, kx]
        A = Wm.reshape(OC, 128, CC, 128, 3, 3).astype(np.float64)
        g = np.stack([
            A[..., 0],
            0.5 * (A[..., 0] + A[..., 1] + A[..., 2]),
            0.5 * (A[..., 0] - A[..., 1] + A[..., 2]),
            A[..., 2],
        ], axis=1)                               # [oc, m, e, cc, c, ky]
        out = g.transpose(0, 4, 1, 5, 3, 2)      # [oc, c, m, ky, cc, e]
        return np.ascontiguousarray(out, dtype=np.float16)

    def bprep(bm):
        return np.ascontiguousarray(bm.reshape(OC, 128).T, dtype=np.float32)

    def brprep(bm):
        return np.ascontiguousarray(bm.reshape(1, E), dtype=np.float16)

    return {
        "wgq": wprep_wino(Wq), "wgk": wprep_wino(Wk), "wv": wprep(Wv),
        "brq": brprep(bq), "brk": brprep(bk), "bv": bprep(bv),
    }


def kernel(feat, Wq, bq, Wk, bk, Wv, bv):
    feat = np.asarray(feat, dtype=np.float32)
    if "nc" not in _CACHE:
        _CACHE["nc"] = _build()
    nc = _CACHE["nc"]

    shared = _prep_shared(np.asarray(Wq, np.float32), np.asarray(bq, np.float32),
                          np.asarray(Wk, np.float32), np.asarray(bk, np.float32),
                          np.asarray(Wv, np.float32), np.asarray(bv, np.float32))

    in_maps = []
    for b in range(B):
        xpad = np.zeros((C, 50, 50), np.float16)
        xpad[:, 1:49, 1:49] = feat[b]
        xpad = np.ascontiguousarray(
            xpad.reshape(CC, 128, 2500).transpose(1, 0, 2)
        )
        in_maps.append({"xpad": xpad, **shared})

    r = bass_utils.run_bass_kernel_spmd(nc, in_maps, list(range(B)))
    out = np.stack(
        [np.ascontiguousarray(
            r.results[b]["out"].reshape(N, E).T).reshape(E, H, W)
         for b in range(B)], axis=0
    )
    return out


# revision 23
# speedup vs baseline: 1.2474x; 1.0230x over previous
"""ConvSA kernel for Trainium2 (8 NeuronCores, data-parallel over batch).

Computes, per batch element b (one per core):
    q/k/v = conv3x3(feat, W{q,k,v}) + b{q,k,v}        # 256 -> 512 ch, SAME pad
    att   = softmax_j(q^T k);  out = v @ att^T + v    # N = 48*48 = 2304

v2 strategy vs v1 (460us): fp16 operands for convs + QK (LDWEIGHTS at
1 cy/row makes matmuls stream-bound instead of weight-load-bound), bf16
for p=exp(s-C) and vT (bf16 keeps fp32 exponent range; fp16 would
underflow rows whose max logit is far below the global shift C).
Attention-value matmul is flipped to produce out^T tiles [i_part, e]:
p slices are the stationary operand, vT streams; the epilogue then
normalizes with a per-partition scalar (activation scale), adds the
residual v^T directly from vT (v_scr DRAM roundtrip eliminated), and
row-sums come from a DVE pairwise tree + one ones-matmul instead of 18
PE matmuls per tile. Output is written transposed [N, E]; the host
untransposes.
"""
import numpy as np
from contextlib import ExitStack

import concourse.bass as bass
import concourse.tile as tile
from concourse import bacc, bass_utils, mybir
from concourse.masks import make_identity

F32 = mybir.dt.float32
F32R = mybir.dt.float32r
F16 = mybir.dt.float16
BF16 = mybir.dt.bfloat16

B, C, H, W = 8, 256, 48, 48
E = 512
N = H * W            # 2304
CC = C // 128        # 2 c-chunks
OC = E // 128        # 4 o-chunks / e-chunks
JC = N // 128        # 18 j-chunks
NT = [(0, 10), (10, 10), (20, 10), (30, 10), (40, 8)]     # conv row tiles
IT = [(0, 512), (512, 512), (1024, 512), (1536, 512), (2048, 256)]  # i tiles

_CACHE = {}


def _build():
    nc = bacc.Bacc("TRN2", target_bir_lowering=False, debug=False, num_devices=B)

    xp_ap = nc.dram_tensor("xpad", [128, CC, 2500], F16, kind="ExternalInput").ap()
    # K/Q conv weights in 1-D Winograd F(2,3) form: [oc, c, m, ky, cc, e]
    wg_aps = {
        cn: nc.dram_tensor(f"wg{cn}", [OC, 128, 4, 3, CC, 128], F16,
                           kind="ExternalInput").ap()
        for cn in "qk"
    }
    br_aps = {
        cn: nc.dram_tensor(f"br{cn}", [1, E], F16, kind="ExternalInput").ap()
        for cn in "qk"
    }
    w_aps = {
        cn: nc.dram_tensor(f"w{cn}", [OC, 128, CC, 9, 128], F16, kind="ExternalInput").ap()
        for cn in "v"
    }
    b_aps = {
        cn: nc.dram_tensor(f"b{cn}", [128, OC], F32, kind="ExternalInput").ap()
        for cn in "v"
    }
    # transposed output: [i-chunk, 128 i, E]
    out_ap = nc.dram_tensor("out", [JC, 128, E], F32, kind="ExternalOutput").ap()

    with tile.TileContext(nc) as tc, ExitStack() as ctx:
        res = ctx.enter_context(tc.tile_pool(name="res", bufs=1))
        k_res = res.tile([128, OC, N], F16, tag="k")
        q_res = res.tile([128, OC, N], F16, tag="q")
        vT = res.tile([128, JC, E], BF16, tag="vT")
        bias_t = {cn: res.tile([128, OC], F32, tag=f"b{cn}", name=f"bias_{cn}")
                  for cn in "v"}
        br_t = {cn: res.tile([1, E], F16, tag=f"br{cn}", name=f"biasr_{cn}")
                for cn in "qk"}
        ones384 = res.tile([1, 512], F16, tag="on384")
        ones_col = res.tile([128, 1], F32R, tag="oc")
        ones_row = res.tile([1, 128], F32R, tag="or")
        ones_one = res.tile([1, 2], F32R, tag="o1")
        negC = res.tile([128, 1], F32, tag="negc")
        ident = res.tile([128, 128], F32R, tag="id")

        # ---------------- conv phase ----------------
        # K/Q: 1-D Winograd F(2,3) along x (1.5x fewer PE MACs).
        # For each output pair (y, 2t / 2t+1):
        #   D0 = x0-x2, D1 = x1+x2, D2 = x2-x1, D3 = x1-x3  (x_k = xpad col 2t+k)
        #   Y_m = sum_{ky,c} G_m(w) * D_m ;  y0 = Y0+Y1+Y2, y1 = Y1-Y2-Y3
        # with G rows {w0, (w0+w1+w2)/2, (w0-w1+w2)/2, w2}; conv bias is
        # injected into the Y1 accumulation via a [1,e]x[1,384] matmul.
        xpad_t = res.tile([128, CC, 50, 50], F16, tag="x")
        with tc.tile_pool(name="xw", bufs=1) as xwp, \
             tc.tile_pool(name="wg", bufs=3) as wgp, \
             tc.tile_pool(name="wtmp", bufs=3) as wtp, \
             tc.tile_pool(name="wps", bufs=2, space="PSUM") as wps:
            xt = xwp.tile([128, 4, CC, 50, 24], F16, tag="xt")

            def emit_xt(cc):
                xr = xpad_t[:, cc].rearrange("p h (x two) -> p h x two", two=2)
                x0 = xr[:, :, 0:24, 0]
                x1 = xr[:, :, 0:24, 1]
                x2 = xr[:, :, 1:25, 0]
                x3 = xr[:, :, 1:25, 1]
                with nc.allow_low_precision(reason="fp16 winograd input tf"):
                    nc.vector.tensor_tensor(xt[:, 0, cc], x0, x2,
                                            mybir.AluOpType.subtract)
                    nc.vector.tensor_tensor(xt[:, 1, cc], x1, x2,
                                            mybir.AluOpType.add)
                    nc.vector.tensor_tensor(xt[:, 2, cc], x2, x1,
                                            mybir.AluOpType.subtract)
                    nc.vector.tensor_tensor(xt[:, 3, cc], x1, x3,
                                            mybir.AluOpType.subtract)

            # head DMA order: xpad cc0, first m-chunk of K weights, rest
            nc.sync.dma_start(
                out=xpad_t[:, 0].rearrange("p h w -> p (h w)"), in_=xp_ap[:, 0])
            wg_k0 = wgp.tile([128, 4, 3, CC, 128], F16, tag="wg", name="wg_k0")
            nc.sync.dma_start(out=wg_k0[:, 0], in_=wg_aps["k"][0][:, 0])
            emit_xt(0)
            nc.sync.dma_start(out=wg_k0[:, 1:4], in_=wg_aps["k"][0][:, 1:4])
            nc.sync.dma_start(
                out=xpad_t[:, 1].rearrange("p h w -> p (h w)"), in_=xp_ap[:, 1])
            emit_xt(1)
            for cn in "kq":
                nc.sync.dma_start(out=br_t[cn], in_=br_aps[cn])
            nc.sync.dma_start(out=bias_t["v"], in_=b_aps["v"])
            ident_raw = xwp.tile([128, 128], F32, tag="idr")
            make_identity(nc, ident_raw)
            nc.vector.tensor_copy(out=ident, in_=ident_raw)
            ones_raw = xwp.tile([128, 1], F32, tag="onr")
            nc.vector.memset(ones_raw, 1.0)
            nc.vector.tensor_copy(out=ones_col, in_=ones_raw)
            ones_raw2 = xwp.tile([1, 128], F32, tag="onr2")
            nc.vector.memset(ones_raw2, 1.0)
            nc.vector.tensor_copy(out=ones_row, in_=ones_raw2)
            nc.vector.tensor_copy(out=ones_one, in_=ones_raw2[:, 0:2])
            nc.vector.memset(ones384, 1.0)

            def wconv(cn, dst, wg_pre=None):
                dstr = dst.rearrange("p o (y x two) -> p o y x two",
                                     y=48, two=2)
                for oc in range(OC):
                    if oc == 0 and wg_pre is not None:
                        wt = wg_pre
                    else:
                        wt = wgp.tile([128, 4, 3, CC, 128], F16, tag="wg")
                        nc.sync.dma_start(out=wt, in_=wg_aps[cn][oc])
                    for th in range(3):
                        mt = [wps.tile([128, 512], F32, tag=f"m{m}",
                                       name=f"m{m}_{cn}_{oc}_{th}")
                              for m in range(4)]
                        for m in range(4):
                            for ky in range(3):
                                for cc in range(CC):
                                    nc.tensor.matmul(
                                        mt[m][:, 0:384],
                                        wt[:, m, ky, cc, :],
                                        xt[:, m, cc, ky:ky + 48,
                                           th * 8:th * 8 + 8],
                                        start=(ky == 0 and cc == 0),
                                        stop=(ky == 2 and cc == 1 and m != 1),
                                    )
                            if m == 1:
                                nc.tensor.matmul(
                                    mt[1][:, 0:384],
                                    br_t[cn][:, oc * 128:(oc + 1) * 128],
                                    ones384[:, 0:384],
                                    start=False, stop=True,
                                )
                        mv = [mt[m][:, 0:384].rearrange(
                            "p (y x) -> p y x", y=48) for m in range(4)]
                        # DVE reads at most one PSUM operand per op, so the
                        # Act engine first stages m1/m2 into SBUF.
                        m1s = wtp.tile([128, 384], F32, tag="m1s",
                                       name=f"m1s_{cn}_{oc}_{th}")
                        nc.scalar.copy(out=m1s, in_=mt[1][:, 0:384])
                        m1sv = m1s.rearrange("p (y x) -> p y x", y=48)
                        m2s = wtp.tile([128, 384], F32, tag="m2s",
                                       name=f"m2s_{cn}_{oc}_{th}")
                        nc.scalar.copy(out=m2s, in_=mt[2][:, 0:384])
                        m2sv = m2s.rearrange("p (y x) -> p y x", y=48)
                        t01 = wtp.tile([128, 384], F32, tag="t01",
                                       name=f"t01_{cn}_{oc}_{th}")
                        t01v = t01.rearrange("p (y x) -> p y x", y=48)
                        t23 = wtp.tile([128, 384], F32, tag="t23",
                                       name=f"t23_{cn}_{oc}_{th}")
                        t23v = t23.rearrange("p (y x) -> p y x", y=48)
                        d0 = dstr[:, oc, :, th * 8:th * 8 + 8, 0]
                        d1 = dstr[:, oc, :, th * 8:th * 8 + 8, 1]
                        with nc.allow_low_precision(reason="winograd out tf"):
                            nc.vector.tensor_tensor(t01v, mv[0], m1sv,
                                                    mybir.AluOpType.add)
                            nc.vector.tensor_tensor(t23v, m2sv, mv[3],
                                                    mybir.AluOpType.add)
                            nc.vector.tensor_tensor(d0, t01v, mv[2],
                                                    mybir.AluOpType.add)
                            nc.vector.tensor_tensor(d1, mv[1], t23v,
                                                    mybir.AluOpType.subtract)

            wconv("k", k_res, wg_pre=wg_k0)
            # prefetch first V-conv weight tile during the Q conv
            w_v0 = res.tile([128, CC, 9, 128], F16, tag="wv0")
            nc.sync.dma_start(out=w_v0, in_=w_aps["v"][0])
            wconv("q", q_res)

        # v conv (direct): per o-chunk, transpose into vT (bf16); residual
        # v^T is read straight out of vT later, no DRAM scratch needed.
        # The global-shift-constant block is emitted after oc0 so its
        # PE<->DVE ping-pong latency hides under the remaining V GEMMs.
        with tc.tile_pool(name="w", bufs=3) as wp, \
             tc.tile_pool(name="vst", bufs=2) as vstp, \
             tc.tile_pool(name="cps", bufs=2, space="PSUM") as cps, \
             tc.tile_pool(name="tps", bufs=2, space="PSUM") as tps, \
             tc.tile_pool(name="mps", bufs=1, space="PSUM") as mps, \
             tc.tile_pool(name="msb", bufs=1) as msb:

            def emit_negc():
                # C = max over i in [0,256) x j in [0,512) of s -- any
                # constant with rowmax-80 <= C <= globalmax+88 keeps exp()
                # in fp32/bf16 range; softmax is shift-invariant so the
                # result is exact.
                mini = mps.tile([128, 4, 256], F32, tag="mini")
                for jc in range(4):
                    for ec in range(OC):
                        nc.tensor.matmul(
                            mini[:, jc, :], k_res[:, ec, jc * 128:(jc + 1) * 128],
                            q_res[:, ec, 0:256], start=(ec == 0), stop=(ec == OC - 1),
                        )
                m1 = msb.tile([128, 1], F32R, tag="m1")
                nc.vector.reduce_max(out=m1, in_=mini, axis=mybir.AxisListType.XY)
                tpm = mps.tile([1, 128], F32R, tag="tpm")
                nc.tensor.transpose(tpm, m1, ident)
                cneg = msb.tile([1, 2], F32R, tag="cn")
                nc.vector.reduce_max(out=cneg[:, 0:1], in_=tpm,
                                     axis=mybir.AxisListType.X, negate=True)
                nc.vector.tensor_copy(out=cneg[:, 1:2], in_=cneg[:, 0:1])
                ncps = mps.tile([128, 2], F32, tag="ncps")
                nc.tensor.matmul(ncps, ones_row, cneg, start=True, stop=True)
                nc.vector.tensor_copy(out=negC, in_=ncps[:, 0:1])

            for oc in range(OC):
                if oc == 0:
                    w_t = w_v0
                else:
                    w_t = wp.tile([128, CC, 9, 128], F16, tag="w")
                    nc.sync.dma_start(out=w_t, in_=w_aps["v"][oc])
                vs = vstp.tile([128, N], F32R, tag="vs")
                for (y0, rr) in NT:
                    ps = cps.tile([128, rr * 48], F32, tag="conv")
                    first = True
                    for cc in range(CC):
                        for ky in range(3):
                            for kx in range(3):
                                rhs = xpad_t[:, cc, y0 + ky:y0 + ky + rr, kx:kx + 48]
                                nc.tensor.matmul(
                                    ps, w_t[:, cc, ky * 3 + kx, :], rhs,
                                    start=first, stop=(cc == CC - 1 and ky == 2 and kx == 2),
                                )
                                first = False
                    nc.scalar.activation(
                        out=vs[:, y0 * 48:(y0 + rr) * 48], in_=ps,
                        func=mybir.ActivationFunctionType.Identity,
                        bias=bias_t["v"][:, oc:oc + 1], scale=1.0,
                    )
                for jc in range(JC):
                    tp = tps.tile([128, 128], F32R, tag="t")
                    nc.tensor.transpose(tp, vs[:, jc * 128:(jc + 1) * 128], ident)
                    # Act engine copies PSUM->SBUF with cast to bf16
                    nc.scalar.copy(out=vT[:, jc, oc * 128:(oc + 1) * 128], in_=tp)
                if oc == 0:
                    emit_negc()

        # ---------------- attention ----------------
        with tc.tile_pool(name="pp", bufs=2) as pp, \
             tc.tile_pool(name="trp", bufs=2) as trp, \
             tc.tile_pool(name="esb", bufs=2) as esb, \
             tc.tile_pool(name="sps", bufs=3, space="PSUM") as sps, \
             tc.tile_pool(name="aps", bufs=3, space="PSUM") as aps, \
             tc.tile_pool(name="rps", bufs=1, space="PSUM") as rps, \
             tc.tile_pool(name="cps2", bufs=1, space="PSUM") as cps2:
            p_tiles = {}

            def emit_qk(t):
                i0, iw = IT[t]
                p_t = pp.tile([128, JC, iw], BF16, tag="p", name=f"p_{t}")
                p_tiles[t] = p_t
                for jc in range(JC):
                    ps = sps.tile([128, iw], F32, tag="s", name=f"s_{t}_{jc}")
                    for ec in range(OC):
                        nc.tensor.matmul(
                            ps, k_res[:, ec, jc * 128:(jc + 1) * 128],
                            q_res[:, ec, i0:i0 + iw],
                            start=(ec == 0), stop=(ec == OC - 1),
                        )
                    nc.scalar.activation(
                        out=p_t[:, jc, :], in_=ps,
                        func=mybir.ActivationFunctionType.Exp,
                        bias=negC[:, 0:1], scale=1.0,
                    )

            def emit_post(t):
                i0, iw = IT[t]
                nsub = iw // 128
                p_t = p_tiles.pop(t)
                # rowsum tree on DVE: 18 = 8+8+2
                t8 = trp.tile([128, 8, iw], F32, tag="t8", name=f"t8_{t}")
                t4 = trp.tile([128, 4, iw], F32, tag="t4", name=f"t4_{t}")
                t2 = trp.tile([128, 2, iw], F32, tag="t2", name=f"t2_{t}")
                tx = trp.tile([128, 1, iw], F32, tag="tx", name=f"tx_{t}")
                t1 = trp.tile([128, iw], F32R, tag="t1", name=f"t1_{t}")
                with nc.allow_low_precision(reason="f32-accurate rowsum tree"):
                    nc.vector.tensor_tensor(t8, p_t[:, 0:8, :], p_t[:, 8:16, :],
                                            mybir.AluOpType.add)
                    nc.vector.tensor_tensor(tx, p_t[:, 16:17, :], p_t[:, 17:18, :],
                                            mybir.AluOpType.add)
                    nc.vector.tensor_tensor(t4, t8[:, 0:4, :], t8[:, 4:8, :],
                                            mybir.AluOpType.add)
                    nc.vector.tensor_tensor(t2, t4[:, 0:2, :], t4[:, 2:4, :],
                                            mybir.AluOpType.add)
                    nc.vector.tensor_tensor(t2[:, 0:1, :], t2[:, 0:1, :], t2[:, 1:2, :],
                                            mybir.AluOpType.add)
                    nc.vector.tensor_tensor(t1, t2[:, 0, :], tx[:, 0, :],
                                            mybir.AluOpType.add)
                # PE order: AV groups 0..nsub-2, then the short rowsum chain
                # (whose DVE inputs finished long ago), then the last AV
                # group; Act scales + epilogue drain in between.
                avs, rcols = {}, {}

                def emit_av(sub):
                    av = aps.tile([128, E], F32, tag="av", name=f"av_{t}_{sub}")
                    avs[sub] = av
                    for jc in range(JC):
                        nc.tensor.matmul(
                            av, p_t[:, jc, sub * 128:(sub + 1) * 128],
                            vT[:, jc, :],
                            start=(jc == 0), stop=(jc == JC - 1),
                        )

                def emit_scale(sub):
                    ic = i0 // 128 + sub
                    o_bf = esb.tile([128, E], BF16, tag="obf", name=f"obf_{t}_{sub}")
                    nc.scalar.activation(
                        out=o_bf, in_=avs[sub],
                        func=mybir.ActivationFunctionType.Copy,
                        bias=0.0, scale=rcols[sub][:, 0:1],
                    )
                    o_t = esb.tile([128, E], F32, tag="o", name=f"o_{t}_{sub}")
                    nc.vector.tensor_tensor(o_t, o_bf, vT[:, ic, :],
                                            mybir.AluOpType.add)
                    nc.sync.dma_start(out=out_ap[ic], in_=o_t)

                for sub in range(nsub - 1):
                    emit_av(sub)

                rs = rps.tile([1, iw], F32, tag="rs", name=f"rs_{t}")
                nc.tensor.matmul(rs, ones_col, t1, start=True, stop=True)
                rs_sb = esb.tile([1, iw], F32R, tag="rsb", name=f"rsb_{t}")
                with nc.allow_low_precision(reason="copy of f32 psum"):
                    nc.vector.tensor_copy(out=rs_sb, in_=rs)
                for sub in range(nsub):
                    # broadcast rowsum slice onto partitions: [1,128]x[1,2]
                    rc_ps = cps2.tile([128, 2], F32, tag="rc", name=f"rc_{t}_{sub}")
                    nc.tensor.matmul(rc_ps, rs_sb[:, sub * 128:(sub + 1) * 128],
                                     ones_one, start=True, stop=True)
                    rcol = esb.tile([128, 2], F32, tag="rcol", name=f"rcol_{t}_{sub}")
                    rcols[sub] = rcol
                    nc.vector.reciprocal(out=rcol, in_=rc_ps)
                for sub in range(nsub - 1):
                    emit_scale(sub)
                emit_av(nsub - 1)
                emit_scale(nsub - 1)

            emit_qk(0)
            for t in range(1, len(IT)):
                emit_qk(t)
                emit_post(t - 1)
            emit_post(len(IT) - 1)

    nc.compile()
    return nc


def _prep_shared(Wq, bq, Wk, bk, Wv, bv):
    def wprep(Wm):
        A = Wm.reshape(OC, 128, CC, 128, 3, 3)
        Bm = A.transpose(0, 3, 2, 4, 5, 1)      # [oc, c, cc, ky, kx, o]
        return np.ascontiguousarray(
            Bm.reshape(OC, 128, CC, 9, 128), dtype=np.float16)

    def wprep_wino(Wm):
        # [oc, e, cc, c, ky, kx]
        A = Wm.reshape(OC, 128, CC, 128, 3, 3).astype(np.float64)
        g = np.stack([
            A[..., 0],
            0.5 * (A[..., 0] + A[..., 1] + A[..., 2]),
            0.5 * (A[..., 0] - A[..., 1] + A[..., 2]),
            A[..., 2],
        ], axis=1)                               # [oc, m, e, cc, c, ky]
        out = g.transpose(0, 4, 1, 5, 3, 2)      # [oc, c, m, ky, cc, e]
        return np.ascontiguousarray(out, dtype=np.float16)

    def bprep(bm):
        return np.ascontiguousarray(bm.reshape(OC, 128).T, dtype=np.float32)

    def brprep(bm):
        return np.ascontiguousarray(bm.reshape(1, E), dtype=np.float16)

    return {
        "wgq": wprep_wino(Wq), "wgk": wprep_wino(Wk), "wv": wprep(Wv),
        "brq": brprep(bq), "brk": brprep(bk), "bv": bprep(bv),
    }


def kernel(feat, Wq, bq, Wk, bk, Wv, bv):
    feat = np.asarray(feat, dtype=np.float32)
    if "nc" not in _CACHE:
        _CACHE["nc"] = _build()
    nc = _CACHE["nc"]

    shared = _prep_shared(np.asarray(Wq, np.float32), np.asarray(bq, np.float32),
                          np.asarray(Wk, np.float32), np.asarray(bk, np.float32),
                          np.asarray(Wv, np.float32), np.asarray(bv, np.float32))

    in_maps = []
    for b in range(B):
        xpad = np.zeros((C, 50, 50), np.float16)
        xpad[:, 1:49, 1:49] = feat[b]
        xpad = np.ascontiguousarray(
            xpad.reshape(CC, 128, 2500).transpose(1, 0, 2)
        )
        in_maps.append({"xpad": xpad, **shared})

    r = bass_utils.run_bass_kernel_spmd(nc, in_maps, list(range(B)))
    out = np.stack(
        [np.ascontiguousarray(
            r.results[b]["out"].reshape(N, E).T).reshape(E, H, W)
         for b in range(B)], axis=0
    )
    return out


# revision 27
# speedup vs baseline: 1.2852x; 1.0303x over previous
"""ConvSA kernel for Trainium2 (8 NeuronCores, data-parallel over batch).

Computes, per batch element b (one per core):
    q/k/v = conv3x3(feat, W{q,k,v}) + b{q,k,v}        # 256 -> 512 ch, SAME pad
    att   = softmax_j(q^T k);  out = v @ att^T + v    # N = 48*48 = 2304

Pipeline (one NeuronCore per batch element):
  * V conv first (direct fp16 matmuls) + PE transposes into vT (bf16);
    its Act-engine PSUM->SBUF copies drain under the K/Q GEMMs, and the
    residual v^T is later read straight out of vT (no DRAM scratch).
  * K/Q convs via 1-D Winograd F(2,3) along x: 1.5x fewer PE MACs; the
    input transform is 8 DVE ops (shared), the output transform is 4 DVE
    ops per (oc, x-third) plus 2 Act staging copies, and the conv bias
    rides the Y1 accumulation as a [1,e]x[1,384] matmul.
  * Attention in the s^T[j,i] orientation with one global shift C
    (computed from a logit sample inside the attention scope, PE-hidden
    under the first QK tile).  p = exp(s-C) stored bf16 (fp32 exponent
    range -- fp16 would underflow rows whose max logit is far below C).
  * AV is flipped: p slices stationary, vT streaming, giving out^T[i,e]
    tiles; normalization is a per-partition activation scale (reciprocal
    of a DVE-tree rowsum broadcast via a tiny [1,128]x[1,2] matmul), and
    the residual is a plain vT slice add.  Output is DMA'd transposed
    [N, E]; the host untransposes.
All matmul operands are fp16/bf16 (1 cy/row streams AND 1 cy/row
LDWEIGHTS, making the PE stream-bound), PSUM stays f32.
"""
import numpy as np
from contextlib import ExitStack

import concourse.bass as bass
import concourse.tile as tile
from concourse import bacc, bass_utils, mybir
from concourse.masks import make_identity

F32 = mybir.dt.float32
F32R = mybir.dt.float32r
F16 = mybir.dt.float16
BF16 = mybir.dt.bfloat16

B, C, H, W = 8, 256, 48, 48
E = 512
N = H * W            # 2304
CC = C // 128        # 2 c-chunks
OC = E // 128        # 4 o-chunks / e-chunks
JC = N // 128        # 18 j-chunks
NT = [(0, 10), (10, 10), (20, 10), (30, 10), (40, 8)]     # conv row tiles
IT = [(0, 512), (512, 512), (1024, 512), (1536, 512), (2048, 256)]  # i tiles

_CACHE = {}


def _build():
    nc = bacc.Bacc("TRN2", target_bir_lowering=False, debug=False, num_devices=B)

    xp_ap = nc.dram_tensor("xpad", [128, CC, 2500], F16, kind="ExternalInput").ap()
    # K/Q conv weights in 1-D Winograd F(2,3) form: [oc, c, m, ky, cc, e]
    wg_aps = {
        cn: nc.dram_tensor(f"wg{cn}", [OC, 128, 4, 3, CC, 128], F16,
                           kind="ExternalInput").ap()
        for cn in "qk"
    }
    br_aps = {
        cn: nc.dram_tensor(f"br{cn}", [1, E], F16, kind="ExternalInput").ap()
        for cn in "qk"
    }
    wv_ap = nc.dram_tensor("wv", [OC, 128, CC, 9, 128], F16,
                           kind="ExternalInput").ap()
    bv_ap = nc.dram_tensor("bv", [128, OC], F32, kind="ExternalInput").ap()
    # transposed output: [i-chunk, 128 i, E]
    out_ap = nc.dram_tensor("out", [JC, 128, E], F32, kind="ExternalOutput").ap()

    with tile.TileContext(nc) as tc, ExitStack() as ctx:
        res = ctx.enter_context(tc.tile_pool(name="res", bufs=1))
        k_res = res.tile([128, OC, N], F16, tag="k")
        q_res = res.tile([128, OC, N], F16, tag="q")
        vT = res.tile([128, JC, E], BF16, tag="vT")
        bias_v = res.tile([128, OC], F32, tag="bv")
        br_t = {cn: res.tile([1, E], F16, tag=f"br{cn}", name=f"biasr_{cn}")
                for cn in "qk"}
        ones384 = res.tile([1, 512], F16, tag="on384")
        ones_col = res.tile([128, 1], F32R, tag="oc")
        ones_row = res.tile([1, 128], F32R, tag="or")
        ones_one = res.tile([1, 2], F32R, tag="o1")
        negC = res.tile([128, 1], F32, tag="negc")
        ident = res.tile([128, 128], F32R, tag="id")
        xpad_t = res.tile([128, CC, 50, 50], F16, tag="x")
        w_v0 = res.tile([128, CC, 9, 128], F16, tag="wv0")

        # ---------------- conv phase ----------------
        with tc.tile_pool(name="xw", bufs=1) as xwp, \
             tc.tile_pool(name="wg", bufs=3) as wgp, \
             tc.tile_pool(name="w", bufs=3) as wp, \
             tc.tile_pool(name="vst", bufs=2) as vstp, \
             tc.tile_pool(name="wtmp", bufs=3) as wtp, \
             tc.tile_pool(name="wps", bufs=2, space="PSUM") as wps:
            # head DMAs: first V-conv row tiles need xpad cc0 rows 0-13
            # and the first V weight tile; everything else has V-conv cover.
            nc.sync.dma_start(
                out=xpad_t[:, 0, 0:14].rearrange("p h w -> p (h w)"),
                in_=xp_ap[:, 0, 0:700],
            )
            nc.sync.dma_start(out=w_v0, in_=wv_ap[0])
            nc.sync.dma_start(
                out=xpad_t[:, 0, 14:50].rearrange("p h w -> p (h w)"),
                in_=xp_ap[:, 0, 700:2500],
            )
            nc.sync.dma_start(
                out=xpad_t[:, 1].rearrange("p h w -> p (h w)"), in_=xp_ap[:, 1])
            nc.sync.dma_start(out=bias_v, in_=bv_ap)
            for cn in "kq":
                nc.sync.dma_start(out=br_t[cn], in_=br_aps[cn])
            ident_raw = xwp.tile([128, 128], F32, tag="idr")
            make_identity(nc, ident_raw)
            nc.vector.tensor_copy(out=ident, in_=ident_raw)
            ones_raw = xwp.tile([128, 1], F32, tag="onr")
            nc.vector.memset(ones_raw, 1.0)
            nc.vector.tensor_copy(out=ones_col, in_=ones_raw)
            ones_raw2 = xwp.tile([1, 128], F32, tag="onr2")
            nc.vector.memset(ones_raw2, 1.0)
            nc.vector.tensor_copy(out=ones_row, in_=ones_raw2)
            nc.vector.tensor_copy(out=ones_one, in_=ones_raw2[:, 0:2])
            nc.vector.memset(ones384, 1.0)

            xt = xwp.tile([128, 4, CC, 50, 24], F16, tag="xt")

            def emit_xt(cc):
                xr = xpad_t[:, cc].rearrange("p h (x two) -> p h x two", two=2)
                x0 = xr[:, :, 0:24, 0]
                x1 = xr[:, :, 0:24, 1]
                x2 = xr[:, :, 1:25, 0]
                x3 = xr[:, :, 1:25, 1]
                with nc.allow_low_precision(reason="fp16 winograd input tf"):
                    nc.vector.tensor_tensor(xt[:, 0, cc], x0, x2,
                                            mybir.AluOpType.subtract)
                    nc.vector.tensor_tensor(xt[:, 1, cc], x1, x2,
                                            mybir.AluOpType.add)
                    nc.vector.tensor_tensor(xt[:, 2, cc], x2, x1,
                                            mybir.AluOpType.subtract)
                    nc.vector.tensor_tensor(xt[:, 3, cc], x1, x3,
                                            mybir.AluOpType.subtract)

            # --- V conv (direct) + transposes into vT; shares the wps
            # PSUM banks (tags m0/m1 for conv psum, m2/m3 for transposes).
            for oc in range(OC):
                if oc == 0:
                    w_t = w_v0
                else:
                    w_t = wp.tile([128, CC, 9, 128], F16, tag="w")
                    nc.sync.dma_start(out=w_t, in_=wv_ap[oc])
                vs = vstp.tile([128, N], F32R, tag="vs")
                for ti, (y0, rr) in enumerate(NT):
                    ps = wps.tile([128, 512], F32, tag=f"m{ti % 2}",
                                  name=f"vps_{oc}_{ti}")
                    first = True
                    for cc in range(CC):
                        for ky in range(3):
                            for kx in range(3):
                                rhs = xpad_t[:, cc, y0 + ky:y0 + ky + rr, kx:kx + 48]
                                nc.tensor.matmul(
                                    ps[:, 0:rr * 48], w_t[:, cc, ky * 3 + kx, :], rhs,
                                    start=first, stop=(cc == CC - 1 and ky == 2 and kx == 2),
                                )
                                first = False
                    nc.scalar.activation(
                        out=vs[:, y0 * 48:(y0 + rr) * 48], in_=ps[:, 0:rr * 48],
                        func=mybir.ActivationFunctionType.Identity,
                        bias=bias_v[:, oc:oc + 1], scale=1.0,
                    )
                for jc in range(JC):
                    tp = wps.tile([128, 128], F32R, tag=f"m{2 + jc % 2}",
                                  name=f"tp_{oc}_{jc}")
                    nc.tensor.transpose(tp, vs[:, jc * 128:(jc + 1) * 128], ident)
                    # Act engine copies PSUM->SBUF with cast to bf16
                    nc.scalar.copy(out=vT[:, jc, oc * 128:(oc + 1) * 128], in_=tp)
                if oc == 0:
                    # winograd input transform + first K weights, under V cover
                    emit_xt(0)
                    emit_xt(1)
                    wg_k0 = wgp.tile([128, 4, 3, CC, 128], F16, tag="wg",
                                     name="wg_k0")
                    nc.sync.dma_start(out=wg_k0, in_=wg_aps["k"][0])

            # --- K/Q convs, 1-D Winograd F(2,3) ---
            def wconv(cn, dst, wg_pre=None):
                dstr = dst.rearrange("p o (y x two) -> p o y x two",
                                     y=48, two=2)
                for oc in range(OC):
                    if oc == 0 and wg_pre is not None:
                        wt = wg_pre
                    else:
                        wt = wgp.tile([128, 4, 3, CC, 128], F16, tag="wg")
                        nc.sync.dma_start(out=wt, in_=wg_aps[cn][oc])
                    for th in range(3):
                        mt = [wps.tile([128, 512], F32, tag=f"m{m}",
                                       name=f"m{m}_{cn}_{oc}_{th}")
                              for m in range(4)]
                        for m in range(4):
                            for ky in range(3):
                                for cc in range(CC):
                                    nc.tensor.matmul(
                                        mt[m][:, 0:384],
                                        wt[:, m, ky, cc, :],
                                        xt[:, m, cc, ky:ky + 48,
                                           th * 8:th * 8 + 8],
                                        start=(ky == 0 and cc == 0),
                                        stop=(ky == 2 and cc == 1 and m != 1),
                                    )
                            if m == 1:
                                nc.tensor.matmul(
                                    mt[1][:, 0:384],
                                    br_t[cn][:, oc * 128:(oc + 1) * 128],
                                    ones384[:, 0:384],
                                    start=False, stop=True,
                                )
                        mv = [mt[m][:, 0:384].rearrange(
                            "p (y x) -> p y x", y=48) for m in range(4)]
                        # DVE reads at most one PSUM operand per op, so the
                        # Act engine first stages m1/m2 into SBUF.
                        m1s = wtp.tile([128, 384], F32, tag="m1s",
                                       name=f"m1s_{cn}_{oc}_{th}")
                        nc.scalar.copy(out=m1s, in_=mt[1][:, 0:384])
                        m1sv = m1s.rearrange("p (y x) -> p y x", y=48)
                        m2s = wtp.tile([128, 384], F32, tag="m2s",
                                       name=f"m2s_{cn}_{oc}_{th}")
                        nc.scalar.copy(out=m2s, in_=mt[2][:, 0:384])
                        m2sv = m2s.rearrange("p (y x) -> p y x", y=48)
                        t01 = wtp.tile([128, 384], F32, tag="t01",
                                       name=f"t01_{cn}_{oc}_{th}")
                        t01v = t01.rearrange("p (y x) -> p y x", y=48)
                        t23 = wtp.tile([128, 384], F32, tag="t23",
                                       name=f"t23_{cn}_{oc}_{th}")
                        t23v = t23.rearrange("p (y x) -> p y x", y=48)
                        d0 = dstr[:, oc, :, th * 8:th * 8 + 8, 0]
                        d1 = dstr[:, oc, :, th * 8:th * 8 + 8, 1]
                        with nc.allow_low_precision(reason="winograd out tf"):
                            nc.vector.tensor_tensor(t01v, mv[0], m1sv,
                                                    mybir.AluOpType.add)
                            nc.vector.tensor_tensor(t23v, m2sv, mv[3],
                                                    mybir.AluOpType.add)
                            nc.vector.tensor_tensor(d0, t01v, mv[2],
                                                    mybir.AluOpType.add)
                            nc.vector.tensor_tensor(d1, mv[1], t23v,
                                                    mybir.AluOpType.subtract)

            wconv("k", k_res, wg_pre=wg_k0)
            wconv("q", q_res)

            # ---- global shift constant C (tail of the conv scope; its
            # PE cost is ~1us, the DVE ping-pong overlaps the last
            # winograd output-transform drain).  C = max over i,j in
            # [0,256)^2 of s -- any constant with rowmax-80 <= C <=
            # globalmax+88 keeps exp() in fp32/bf16 range; softmax is
            # shift-invariant so the result is exact.
            mini_t = wps.tile([128, 512], F32, tag="m0", name="mini")
            mini = mini_t.rearrange("p (j x) -> p j x", j=2)
            for jc in range(2):
                for ec in range(OC):
                    nc.tensor.matmul(
                        mini[:, jc, :], k_res[:, ec, jc * 128:(jc + 1) * 128],
                        q_res[:, ec, 0:256], start=(ec == 0), stop=(ec == OC - 1),
                    )
            m1 = wtp.tile([128, 1], F32R, tag="m1n")
            nc.vector.reduce_max(out=m1, in_=mini, axis=mybir.AxisListType.XY)
            tpm = wps.tile([1, 128], F32R, tag="m2", name="tpm")
            nc.tensor.transpose(tpm, m1, ident)
            cneg = wtp.tile([1, 2], F32R, tag="cnn")
            nc.vector.reduce_max(out=cneg[:, 0:1], in_=tpm,
                                 axis=mybir.AxisListType.X, negate=True)
            nc.vector.tensor_copy(out=cneg[:, 1:2], in_=cneg[:, 0:1])
            ncps = wps.tile([128, 2], F32, tag="m3", name="ncps")
            nc.tensor.matmul(ncps, ones_row, cneg, start=True, stop=True)
            nc.vector.tensor_copy(out=negC, in_=ncps[:, 0:1])

        # ---------------- attention ----------------
        with tc.tile_pool(name="pp", bufs=2) as pp, \
             tc.tile_pool(name="trp", bufs=2) as trp, \
             tc.tile_pool(name="esb", bufs=2) as esb, \
             tc.tile_pool(name="sps", bufs=3, space="PSUM") as sps, \
             tc.tile_pool(name="aps", bufs=3, space="PSUM") as aps, \
             tc.tile_pool(name="rps", bufs=1, space="PSUM") as rps, \
             tc.tile_pool(name="cps2", bufs=1, space="PSUM") as cps2:

            p_tiles = {}

            def emit_qk(t):
                i0, iw = IT[t]
                p_t = pp.tile([128, JC, iw], BF16, tag="p", name=f"p_{t}")
                p_tiles[t] = p_t
                for jc in range(JC):
                    ps = sps.tile([128, iw], F32, tag="s", name=f"s_{t}_{jc}")
                    for ec in range(OC):
                        nc.tensor.matmul(
                            ps, k_res[:, ec, jc * 128:(jc + 1) * 128],
                            q_res[:, ec, i0:i0 + iw],
                            start=(ec == 0), stop=(ec == OC - 1),
                        )
                    nc.scalar.activation(
                        out=p_t[:, jc, :], in_=ps,
                        func=mybir.ActivationFunctionType.Exp,
                        bias=negC[:, 0:1], scale=1.0,
                    )

            def emit_post(t):
                i0, iw = IT[t]
                nsub = iw // 128
                p_t = p_tiles.pop(t)
                # rowsum tree on DVE: 18 = 8+8+2, all same-dtype pairs
                t8 = trp.tile([128, 8, iw], F32, tag="t8", name=f"t8_{t}")
                t4 = trp.tile([128, 4, iw], F32, tag="t4", name=f"t4_{t}")
                t2 = trp.tile([128, 2, iw], F32, tag="t2", name=f"t2_{t}")
                tx = trp.tile([128, 1, iw], F32, tag="tx", name=f"tx_{t}")
                t1 = trp.tile([128, iw], F32R, tag="t1", name=f"t1_{t}")
                with nc.allow_low_precision(reason="f32-accurate rowsum tree"):
                    nc.vector.tensor_tensor(t8, p_t[:, 0:8, :], p_t[:, 8:16, :],
                                            mybir.AluOpType.add)
                    nc.vector.tensor_tensor(tx, p_t[:, 16:17, :], p_t[:, 17:18, :],
                                            mybir.AluOpType.add)
                    nc.vector.tensor_tensor(t4, t8[:, 0:4, :], t8[:, 4:8, :],
                                            mybir.AluOpType.add)
                    nc.vector.tensor_tensor(t2, t4[:, 0:2, :], t4[:, 2:4, :],
                                            mybir.AluOpType.add)
                    nc.vector.tensor_tensor(t2[:, 0:1, :], t2[:, 0:1, :], t2[:, 1:2, :],
                                            mybir.AluOpType.add)
                    nc.vector.tensor_tensor(t1, t2[:, 0, :], tx[:, 0, :],
                                            mybir.AluOpType.add)
                avs, rcols = {}, {}

                def emit_av(sub):
                    av = aps.tile([128, E], F32, tag="av", name=f"av_{t}_{sub}")
                    avs[sub] = av
                    for jc in range(JC):
                        nc.tensor.matmul(
                            av, p_t[:, jc, sub * 128:(sub + 1) * 128],
                            vT[:, jc, :],
                            start=(jc == 0), stop=(jc == JC - 1),
                        )

                def emit_scale(sub):
                    ic = i0 // 128 + sub
                    o_bf = esb.tile([128, E], BF16, tag="obf", name=f"obf_{t}_{sub}")
                    nc.scalar.activation(
                        out=o_bf, in_=avs[sub],
                        func=mybir.ActivationFunctionType.Copy,
                        bias=0.0, scale=rcols[sub][:, 0:1],
                    )
                    o_t = esb.tile([128, E], F32, tag="o", name=f"o_{t}_{sub}")
                    nc.vector.tensor_tensor(o_t, o_bf, vT[:, ic, :],
                                            mybir.AluOpType.add)
                    nc.sync.dma_start(out=out_ap[ic], in_=o_t)

                for sub in range(nsub - 1):
                    emit_av(sub)

                rs = rps.tile([1, iw], F32, tag="rs", name=f"rs_{t}")
                nc.tensor.matmul(rs, ones_col, t1, start=True, stop=True)
                rs_sb = esb.tile([1, iw], F32R, tag="rsb", name=f"rsb_{t}")
                with nc.allow_low_precision(reason="copy of f32 psum"):
                    nc.vector.tensor_copy(out=rs_sb, in_=rs)
                for sub in range(nsub):
                    # broadcast rowsum slice onto partitions: [1,128]x[1,2]
                    rc_ps = cps2.tile([128, 2], F32, tag="rc", name=f"rc_{t}_{sub}")
                    nc.tensor.matmul(rc_ps, rs_sb[:, sub * 128:(sub + 1) * 128],
                                     ones_one, start=True, stop=True)
                    rcol = esb.tile([128, 2], F32, tag="rcol", name=f"rcol_{t}_{sub}")
                    rcols[sub] = rcol
                    nc.vector.reciprocal(out=rcol, in_=rc_ps)
                for sub in range(nsub - 1):
                    emit_scale(sub)
                emit_av(nsub - 1)
                emit_scale(nsub - 1)

            emit_qk(0)
            for t in range(1, len(IT)):
                emit_qk(t)
                emit_post(t - 1)
            emit_post(len(IT) - 1)

    nc.compile()
    return nc


def _prep_shared(Wq, bq, Wk, bk, Wv, bv):
    def wprep(Wm):
        A = Wm.reshape(OC, 128, CC, 128, 3, 3)
        Bm = A.transpose(0, 3, 2, 4, 5, 1)      # [oc, c, cc, ky, kx, o]
        return np.ascontiguousarray(
            Bm.reshape(OC, 128, CC, 9, 128), dtype=np.float16)

    def wprep_wino(Wm):
        # [oc, e, cc, c, ky, kx]
        A = Wm.reshape(OC, 128, CC, 128, 3, 3).astype(np.float64)
        g = np.stack([
            A[..., 0],
            0.5 * (A[..., 0] + A[..., 1] + A[..., 2]),
            0.5 * (A[..., 0] - A[..., 1] + A[..., 2]),
            A[..., 2],
        ], axis=1)                               # [oc, m, e, cc, c, ky]
        out = g.transpose(0, 4, 1, 5, 3, 2)      # [oc, c, m, ky, cc, e]
        return np.ascontiguousarray(out, dtype=np.float16)

    def bprep(bm):
        return np.ascontiguousarray(bm.reshape(OC, 128).T, dtype=np.float32)

    def brprep(bm):
        return np.ascontiguousarray(bm.reshape(1, E), dtype=np.float16)

    return {
        "wgq": wprep_wino(Wq), "wgk": wprep_wino(Wk), "wv": wprep(Wv),
        "brq": brprep(bq), "brk": brprep(bk), "bv": bprep(bv),
    }


def kernel(feat, Wq, bq, Wk, bk, Wv, bv):
    feat = np.asarray(feat, dtype=np.float32)
    if "nc" not in _CACHE:
        _CACHE["nc"] = _build()
    nc = _CACHE["nc"]

    shared = _prep_shared(np.asarray(Wq, np.float32), np.asarray(bq, np.float32),
                          np.asarray(Wk, np.float32), np.asarray(bk, np.float32),
                          np.asarray(Wv, np.float32), np.asarray(bv, np.float32))

    in_maps = []
    for b in range(B):
        xpad = np.zeros((C, 50, 50), np.float16)
        xpad[:, 1:49, 1:49] = feat[b]
        xpad = np.ascontiguousarray(
            xpad.reshape(CC, 128, 2500).transpose(1, 0, 2)
        )
        in_maps.append({"xpad": xpad, **shared})

    r = bass_utils.run_bass_kernel_spmd(nc, in_maps, list(range(B)))
    out = np.stack(
        [np.ascontiguousarray(
            r.results[b]["out"].reshape(N, E).T).reshape(E, H, W)
         for b in range(B)], axis=0
    )
    return out


# revision 30
# speedup vs baseline: 1.3040x; 1.0146x over previous
"""ConvSA kernel for Trainium2 (8 NeuronCores, data-parallel over batch).

Computes, per batch element b (one per core):
    q/k/v = conv3x3(feat, W{q,k,v}) + b{q,k,v}        # 256 -> 512 ch, SAME pad
    att   = softmax_j(q^T k);  out = v @ att^T + v    # N = 48*48 = 2304

Pipeline (one NeuronCore per batch element):
  * V conv first (direct fp16 matmuls) + PE transposes into vT (bf16);
    its Act-engine PSUM->SBUF copies drain under the K/Q GEMMs, and the
    residual v^T is later read straight out of vT (no DRAM scratch).
  * K/Q convs via 1-D Winograd F(2,3) along x: 1.5x fewer PE MACs; the
    input transform is 8 DVE ops (shared), the output transform is 4 DVE
    ops per (oc, x-third) plus 2 Act staging copies, and the conv bias
    rides the Y1 accumulation as a [1,e]x[1,384] matmul.
  * Attention in the s^T[j,i] orientation with one global shift C
    (computed from a logit sample inside the attention scope, PE-hidden
    under the first QK tile).  p = exp(s-C) stored bf16 (fp32 exponent
    range -- fp16 would underflow rows whose max logit is far below C).
  * AV is flipped: p slices stationary, vT streaming, giving out^T[i,e]
    tiles; normalization is a per-partition activation scale (reciprocal
    of a DVE-tree rowsum broadcast via a tiny [1,128]x[1,2] matmul), and
    the residual is a plain vT slice add.  Output is DMA'd transposed
    [N, E]; the host untransposes.
All matmul operands are fp16/bf16 (1 cy/row streams AND 1 cy/row
LDWEIGHTS, making the PE stream-bound), PSUM stays f32.
"""
import numpy as np
from contextlib import ExitStack

import concourse.bass as bass
import concourse.tile as tile
from concourse import bacc, bass_utils, mybir
from concourse.masks import make_identity

F32 = mybir.dt.float32
F32R = mybir.dt.float32r
F16 = mybir.dt.float16
BF16 = mybir.dt.bfloat16

B, C, H, W = 8, 256, 48, 48
E = 512
N = H * W            # 2304
CC = C // 128        # 2 c-chunks
OC = E // 128        # 4 o-chunks / e-chunks
JC = N // 128        # 18 j-chunks
NT = [(0, 10), (10, 10), (20, 10), (30, 10), (40, 8)]     # conv row tiles
IT = [(0, 512), (512, 512), (1024, 512), (1536, 512), (2048, 256)]  # i tiles

_CACHE = {}


def _build():
    nc = bacc.Bacc("TRN2", target_bir_lowering=False, debug=False, num_devices=B)

    xp_ap = nc.dram_tensor("xpad", [128, CC, 2500], F16, kind="ExternalInput").ap()
    # K/Q conv weights in 1-D Winograd F(2,3) form: [oc, c, m, ky, cc, e]
    wg_aps = {
        cn: nc.dram_tensor(f"wg{cn}", [OC, 128, 4, 3, CC, 128], F16,
                           kind="ExternalInput").ap()
        for cn in "qk"
    }
    br_aps = {
        cn: nc.dram_tensor(f"br{cn}", [1, E], F16, kind="ExternalInput").ap()
        for cn in "qk"
    }
    wv_ap = nc.dram_tensor("wv", [OC, 128, CC, 9, 128], F16,
                           kind="ExternalInput").ap()
    bv_ap = nc.dram_tensor("bv", [128, OC], F32, kind="ExternalInput").ap()
    # transposed output: [i-chunk, 128 i, E]
    out_ap = nc.dram_tensor("out", [JC, 128, E], F32, kind="ExternalOutput").ap()

    with tile.TileContext(nc) as tc, ExitStack() as ctx:
        res = ctx.enter_context(tc.tile_pool(name="res", bufs=1))
        k_res = res.tile([128, OC, N], F16, tag="k")
        q_res = res.tile([128, OC, N], F16, tag="q")
        vT = res.tile([128, JC, E], BF16, tag="vT")
        bias_v = res.tile([128, OC], F32, tag="bv")
        br_t = {cn: res.tile([1, E], F16, tag=f"br{cn}", name=f"biasr_{cn}")
                for cn in "qk"}
        ones384 = res.tile([1, 512], F16, tag="on384")
        ones_col = res.tile([128, 1], F32R, tag="oc")
        ones_row = res.tile([1, 128], F32R, tag="or")
        ones_one = res.tile([1, 2], F32R, tag="o1")
        negC = res.tile([128, 1], F32, tag="negc")
        ident = res.tile([128, 128], F32R, tag="id")
        xpad_t = res.tile([128, CC, 50, 50], F16, tag="x")
        w_v0 = res.tile([128, CC, 9, 128], F16, tag="wv0")

        # single PSUM pool for the whole kernel (8 banks via 4 tags x 2
        # bufs) -- no PSUM pool-transition barriers between phases
        wps = ctx.enter_context(tc.tile_pool(name="wps", bufs=2, space="PSUM"))

        # ---------------- conv phase ----------------
        with tc.tile_pool(name="xw", bufs=1) as xwp, \
             tc.tile_pool(name="wg", bufs=3) as wgp, \
             tc.tile_pool(name="w", bufs=3) as wp, \
             tc.tile_pool(name="vst", bufs=2) as vstp, \
             tc.tile_pool(name="wtmp", bufs=3) as wtp:
            # head DMAs: first V-conv row tiles need xpad cc0 rows 0-13
            # (sync/SP queue) and the first V weight tile (Act queue, in
            # parallel); everything else has V-conv cover.
            nc.sync.dma_start(
                out=xpad_t[:, 0, 0:14].rearrange("p h w -> p (h w)"),
                in_=xp_ap[:, 0, 0:700],
            )
            nc.scalar.dma_start(out=w_v0, in_=wv_ap[0])
            nc.sync.dma_start(
                out=xpad_t[:, 0, 14:50].rearrange("p h w -> p (h w)"),
                in_=xp_ap[:, 0, 700:2500],
            )
            nc.sync.dma_start(
                out=xpad_t[:, 1].rearrange("p h w -> p (h w)"), in_=xp_ap[:, 1])
            nc.scalar.dma_start(out=bias_v, in_=bv_ap)
            for cn in "kq":
                nc.scalar.dma_start(out=br_t[cn], in_=br_aps[cn])
            ident_raw = xwp.tile([128, 128], F32, tag="idr")
            make_identity(nc, ident_raw)
            nc.vector.tensor_copy(out=ident, in_=ident_raw)
            ones_raw = xwp.tile([128, 1], F32, tag="onr")
            nc.vector.memset(ones_raw, 1.0)
            nc.vector.tensor_copy(out=ones_col, in_=ones_raw)
            ones_raw2 = xwp.tile([1, 128], F32, tag="onr2")
            nc.vector.memset(ones_raw2, 1.0)
            nc.vector.tensor_copy(out=ones_row, in_=ones_raw2)
            nc.vector.tensor_copy(out=ones_one, in_=ones_raw2[:, 0:2])
            nc.vector.memset(ones384, 1.0)

            xt = xwp.tile([128, 4, CC, 50, 24], F16, tag="xt")

            def emit_xt(cc):
                xr = xpad_t[:, cc].rearrange("p h (x two) -> p h x two", two=2)
                x0 = xr[:, :, 0:24, 0]
                x1 = xr[:, :, 0:24, 1]
                x2 = xr[:, :, 1:25, 0]
                x3 = xr[:, :, 1:25, 1]
                with nc.allow_low_precision(reason="fp16 winograd input tf"):
                    nc.vector.tensor_tensor(xt[:, 0, cc], x0, x2,
                                            mybir.AluOpType.subtract)
                    nc.vector.tensor_tensor(xt[:, 1, cc], x1, x2,
                                            mybir.AluOpType.add)
                    nc.vector.tensor_tensor(xt[:, 2, cc], x2, x1,
                                            mybir.AluOpType.subtract)
                    nc.vector.tensor_tensor(xt[:, 3, cc], x1, x3,
                                            mybir.AluOpType.subtract)

            # --- V conv (direct) + transposes into vT; shares the wps
            # PSUM banks (tags m0/m1 for conv psum, m2/m3 for transposes).
            for oc in range(OC):
                if oc == 0:
                    w_t = w_v0
                else:
                    w_t = wp.tile([128, CC, 9, 128], F16, tag="w")
                    nc.sync.dma_start(out=w_t, in_=wv_ap[oc])
                vs = vstp.tile([128, N], F32R, tag="vs")
                for ti, (y0, rr) in enumerate(NT):
                    ps = wps.tile([128, 512], F32, tag=f"m{ti % 2}",
                                  name=f"vps_{oc}_{ti}")
                    first = True
                    for cc in range(CC):
                        for ky in range(3):
                            for kx in range(3):
                                rhs = xpad_t[:, cc, y0 + ky:y0 + ky + rr, kx:kx + 48]
                                nc.tensor.matmul(
                                    ps[:, 0:rr * 48], w_t[:, cc, ky * 3 + kx, :], rhs,
                                    start=first, stop=(cc == CC - 1 and ky == 2 and kx == 2),
                                )
                                first = False
                    nc.scalar.activation(
                        out=vs[:, y0 * 48:(y0 + rr) * 48], in_=ps[:, 0:rr * 48],
                        func=mybir.ActivationFunctionType.Identity,
                        bias=bias_v[:, oc:oc + 1], scale=1.0,
                    )
                for jc in range(JC):
                    tp = wps.tile([128, 128], F32R, tag=f"m{2 + jc % 2}",
                                  name=f"tp_{oc}_{jc}")
                    nc.tensor.transpose(tp, vs[:, jc * 128:(jc + 1) * 128], ident)
                    # Act engine copies PSUM->SBUF with cast to bf16
                    nc.scalar.copy(out=vT[:, jc, oc * 128:(oc + 1) * 128], in_=tp)
                if oc == 0:
                    # winograd input transform + first K weights, under V cover
                    emit_xt(0)
                    emit_xt(1)
                    wg_k0 = wgp.tile([128, 4, 3, CC, 128], F16, tag="wg",
                                     name="wg_k0")
                    nc.sync.dma_start(out=wg_k0, in_=wg_aps["k"][0])

            # --- K/Q convs, 1-D Winograd F(2,3) ---
            def wconv(cn, dst, wg_pre=None):
                dstr = dst.rearrange("p o (y x two) -> p o y x two",
                                     y=48, two=2)
                for oc in range(OC):
                    if oc == 0 and wg_pre is not None:
                        wt = wg_pre
                    else:
                        wt = wgp.tile([128, 4, 3, CC, 128], F16, tag="wg")
                        nc.sync.dma_start(out=wt, in_=wg_aps[cn][oc])
                    for th in range(3):
                        mt = [wps.tile([128, 512], F32, tag=f"m{m}",
                                       name=f"m{m}_{cn}_{oc}_{th}")
                              for m in range(4)]
                        for m in range(4):
                            for ky in range(3):
                                for cc in range(CC):
                                    nc.tensor.matmul(
                                        mt[m][:, 0:384],
                                        wt[:, m, ky, cc, :],
                                        xt[:, m, cc, ky:ky + 48,
                                           th * 8:th * 8 + 8],
                                        start=(ky == 0 and cc == 0),
                                        stop=(ky == 2 and cc == 1 and m != 1),
                                    )
                            if m == 1:
                                nc.tensor.matmul(
                                    mt[1][:, 0:384],
                                    br_t[cn][:, oc * 128:(oc + 1) * 128],
                                    ones384[:, 0:384],
                                    start=False, stop=True,
                                )
                        mv = [mt[m][:, 0:384].rearrange(
                            "p (y x) -> p y x", y=48) for m in range(4)]
                        # DVE reads at most one PSUM operand per op, so the
                        # Act engine first stages m1/m2 into SBUF.
                        m1s = wtp.tile([128, 384], F32, tag="m1s",
                                       name=f"m1s_{cn}_{oc}_{th}")
                        nc.scalar.copy(out=m1s, in_=mt[1][:, 0:384])
                        m1sv = m1s.rearrange("p (y x) -> p y x", y=48)
                        m2s = wtp.tile([128, 384], F32, tag="m2s",
                                       name=f"m2s_{cn}_{oc}_{th}")
                        nc.scalar.copy(out=m2s, in_=mt[2][:, 0:384])
                        m2sv = m2s.rearrange("p (y x) -> p y x", y=48)
                        t01 = wtp.tile([128, 384], F32, tag="t01",
                                       name=f"t01_{cn}_{oc}_{th}")
                        t01v = t01.rearrange("p (y x) -> p y x", y=48)
                        t23 = wtp.tile([128, 384], F32, tag="t23",
                                       name=f"t23_{cn}_{oc}_{th}")
                        t23v = t23.rearrange("p (y x) -> p y x", y=48)
                        d0 = dstr[:, oc, :, th * 8:th * 8 + 8, 0]
                        d1 = dstr[:, oc, :, th * 8:th * 8 + 8, 1]
                        with nc.allow_low_precision(reason="winograd out tf"):
                            nc.vector.tensor_tensor(t01v, mv[0], m1sv,
                                                    mybir.AluOpType.add)
                            nc.vector.tensor_tensor(t23v, m2sv, mv[3],
                                                    mybir.AluOpType.add)
                            nc.vector.tensor_tensor(d0, t01v, mv[2],
                                                    mybir.AluOpType.add)
                            nc.vector.tensor_tensor(d1, mv[1], t23v,
                                                    mybir.AluOpType.subtract)

            wconv("k", k_res, wg_pre=wg_k0)
            wconv("q", q_res)

            # ---- global shift constant C (tail of the conv scope; its
            # PE cost is ~1us, the DVE ping-pong overlaps the last
            # winograd output-transform drain).  C = max over i,j in
            # [0,256)^2 of s -- any constant with rowmax-80 <= C <=
            # globalmax+88 keeps exp() in fp32/bf16 range; softmax is
            # shift-invariant so the result is exact.
            mini_t = wps.tile([128, 512], F32, tag="m0", name="mini")
            mini = mini_t.rearrange("p (j x) -> p j x", j=2)
            for jc in range(2):
                for ec in range(OC):
                    nc.tensor.matmul(
                        mini[:, jc, :], k_res[:, ec, jc * 128:(jc + 1) * 128],
                        q_res[:, ec, 0:256], start=(ec == 0), stop=(ec == OC - 1),
                    )
            m1 = wtp.tile([128, 1], F32R, tag="m1n")
            nc.vector.reduce_max(out=m1, in_=mini, axis=mybir.AxisListType.XY)
            tpm = wps.tile([1, 128], F32R, tag="m2", name="tpm")
            nc.tensor.transpose(tpm, m1, ident)
            cneg = wtp.tile([1, 2], F32R, tag="cnn")
            nc.vector.reduce_max(out=cneg[:, 0:1], in_=tpm,
                                 axis=mybir.AxisListType.X, negate=True)
            nc.vector.tensor_copy(out=cneg[:, 1:2], in_=cneg[:, 0:1])
            ncps = wps.tile([128, 2], F32, tag="m3", name="ncps")
            nc.tensor.matmul(ncps, ones_row, cneg, start=True, stop=True)
            nc.vector.tensor_copy(out=negC, in_=ncps[:, 0:1])

        # ---------------- attention ----------------
        with tc.tile_pool(name="pp", bufs=2) as pp, \
             tc.tile_pool(name="trp", bufs=2) as trp, \
             tc.tile_pool(name="esb", bufs=2) as esb:

            p_tiles = {}

            def emit_qk(t):
                i0, iw = IT[t]
                p_t = pp.tile([128, JC, iw], BF16, tag="p", name=f"p_{t}")
                p_tiles[t] = p_t
                for jc in range(JC):
                    ps = wps.tile([128, 512], F32, tag=f"m{jc % 2}",
                                  name=f"s_{t}_{jc}")
                    for ec in range(OC):
                        nc.tensor.matmul(
                            ps[:, 0:iw], k_res[:, ec, jc * 128:(jc + 1) * 128],
                            q_res[:, ec, i0:i0 + iw],
                            start=(ec == 0), stop=(ec == OC - 1),
                        )
                    nc.scalar.activation(
                        out=p_t[:, jc, :], in_=ps[:, 0:iw],
                        func=mybir.ActivationFunctionType.Exp,
                        bias=negC[:, 0:1], scale=1.0,
                    )

            def emit_post(t):
                i0, iw = IT[t]
                nsub = iw // 128
                p_t = p_tiles.pop(t)
                # rowsum tree on DVE: 18 = 8+8+2, all same-dtype pairs
                t8 = trp.tile([128, 8, iw], F32, tag="t8", name=f"t8_{t}")
                t4 = trp.tile([128, 4, iw], F32, tag="t4", name=f"t4_{t}")
                t2 = trp.tile([128, 2, iw], F32, tag="t2", name=f"t2_{t}")
                tx = trp.tile([128, 1, iw], F32, tag="tx", name=f"tx_{t}")
                t1 = trp.tile([128, iw], F32R, tag="t1", name=f"t1_{t}")
                with nc.allow_low_precision(reason="f32-accurate rowsum tree"):
                    nc.vector.tensor_tensor(t8, p_t[:, 0:8, :], p_t[:, 8:16, :],
                                            mybir.AluOpType.add)
                    nc.vector.tensor_tensor(tx, p_t[:, 16:17, :], p_t[:, 17:18, :],
                                            mybir.AluOpType.add)
                    nc.vector.tensor_tensor(t4, t8[:, 0:4, :], t8[:, 4:8, :],
                                            mybir.AluOpType.add)
                    nc.vector.tensor_tensor(t2, t4[:, 0:2, :], t4[:, 2:4, :],
                                            mybir.AluOpType.add)
                    nc.vector.tensor_tensor(t2[:, 0:1, :], t2[:, 0:1, :], t2[:, 1:2, :],
                                            mybir.AluOpType.add)
                    nc.vector.tensor_tensor(t1, t2[:, 0, :], tx[:, 0, :],
                                            mybir.AluOpType.add)
                avs, rcols = {}, {}

                def emit_av(sub):
                    av = wps.tile([128, E], F32, tag="m2", name=f"av_{t}_{sub}")
                    avs[sub] = av
                    for jc in range(JC):
                        nc.tensor.matmul(
                            av, p_t[:, jc, sub * 128:(sub + 1) * 128],
                            vT[:, jc, :],
                            start=(jc == 0), stop=(jc == JC - 1),
                        )

                def emit_scale(sub, split=1):
                    ic = i0 // 128 + sub
                    hw_ = E // split
                    for h in range(split):
                        sl = slice(h * hw_, (h + 1) * hw_)
                        o_bf = esb.tile([128, E], BF16, tag="obf",
                                        name=f"obf_{t}_{sub}_{h}")
                        nc.scalar.activation(
                            out=o_bf[:, sl], in_=avs[sub][:, sl],
                            func=mybir.ActivationFunctionType.Copy,
                            bias=0.0, scale=rcols[sub][:, 0:1],
                        )
                        o_t = esb.tile([128, E], F32, tag="o",
                                       name=f"o_{t}_{sub}_{h}")
                        nc.vector.tensor_tensor(o_t[:, sl], o_bf[:, sl],
                                                vT[:, ic, sl],
                                                mybir.AluOpType.add)
                        nc.sync.dma_start(out=out_ap[ic][:, sl], in_=o_t[:, sl])

                for sub in range(nsub - 1):
                    emit_av(sub)

                rs = wps.tile([1, 512], F32, tag="m3", name=f"rs_{t}")
                nc.tensor.matmul(rs[:, 0:iw], ones_col, t1, start=True, stop=True)
                rs_sb = esb.tile([1, iw], F32R, tag="rsb", name=f"rsb_{t}")
                with nc.allow_low_precision(reason="copy of f32 psum"):
                    nc.vector.tensor_copy(out=rs_sb, in_=rs[:, 0:iw])
                for sub in range(nsub):
                    # broadcast rowsum slice onto partitions: [1,128]x[1,2]
                    rc_ps = wps.tile([128, 2], F32, tag="m3", name=f"rc_{t}_{sub}")
                    nc.tensor.matmul(rc_ps, rs_sb[:, sub * 128:(sub + 1) * 128],
                                     ones_one, start=True, stop=True)
                    rcol = esb.tile([128, 2], F32, tag="rcol", name=f"rcol_{t}_{sub}")
                    rcols[sub] = rcol
                    nc.vector.reciprocal(out=rcol, in_=rc_ps)
                for sub in range(nsub - 1):
                    emit_scale(sub)
                emit_av(nsub - 1)
                emit_scale(nsub - 1, split=(2 if t == len(IT) - 1 else 1))

            emit_qk(0)
            for t in range(1, len(IT)):
                emit_qk(t)
                emit_post(t - 1)
            emit_post(len(IT) - 1)

    nc.compile()
    return nc


def _prep_shared(Wq, bq, Wk, bk, Wv, bv):
    def wprep(Wm):
        A = Wm.reshape(OC, 128, CC, 128, 3, 3)
        Bm = A.transpose(0, 3, 2, 4, 5, 1)      # [oc, c, cc, ky, kx, o]
        return np.ascontiguousarray(
            Bm.reshape(OC, 128, CC, 9, 128), dtype=np.float16)

    def wprep_wino(Wm):
        # [oc, e, cc, c, ky, kx]
        A = Wm.reshape(OC, 128, CC, 128, 3, 3).astype(np.float64)
        g = np.stack([
            A[..., 0],
            0.5 * (A[..., 0] + A[..., 1] + A[..., 2]),
            0.5 * (A[..., 0] - A[..., 1] + A[..., 2]),
            A[..., 2],
        ], axis=1)                               # [oc, m, e, cc, c, ky]
        out = g.transpose(0, 4, 1, 5, 3, 2)      # [oc, c, m, ky, cc, e]
        return np.ascontiguousarray(out, dtype=np.float16)

    def bprep(bm):
        return np.ascontiguousarray(bm.reshape(OC, 128).T, dtype=np.float32)

    def brprep(bm):
        return np.ascontiguousarray(bm.reshape(1, E), dtype=np.float16)

    return {
        "wgq": wprep_wino(Wq), "wgk": wprep_wino(Wk), "wv": wprep(Wv),
        "brq": brprep(bq), "brk": brprep(bk), "bv": bprep(bv),
    }


def kernel(feat, Wq, bq, Wk, bk, Wv, bv):
    feat = np.asarray(feat, dtype=np.float32)
    if "nc" not in _CACHE:
        _CACHE["nc"] = _build()
    nc = _CACHE["nc"]

    shared = _prep_shared(np.asarray(Wq, np.float32), np.asarray(bq, np.float32),
                          np.asarray(Wk, np.float32), np.asarray(bk, np.float32),
                          np.asarray(Wv, np.float32), np.asarray(bv, np.float32))

    in_maps = []
    for b in range(B):
        xpad = np.zeros((C, 50, 50), np.float16)
        xpad[:, 1:49, 1:49] = feat[b]
        xpad = np.ascontiguousarray(
            xpad.reshape(CC, 128, 2500).transpose(1, 0, 2)
        )
        in_maps.append({"xpad": xpad, **shared})

    r = bass_utils.run_bass_kernel_spmd(nc, in_maps, list(range(B)))
    out = np.stack(
        [np.ascontiguousarray(
            r.results[b]["out"].reshape(N, E).T).reshape(E, H, W)
         for b in range(B)], axis=0
    )
    return out
